# revision 63
# baseline (speedup 1.0000x reference)
"""Trainium2 Bass kernel for a dense transformer block (PreNorm attn + PreNorm MLP).

Sharding (8 cores, collective-free): core c -> batch b = c//2, sequence half
h = c%2.  Each core computes K/V for the full 2048-token sequence of its batch
element (redundant across the core pair) but Q/attention/FFN only for its own
1024 tokens.  The host permutes each core's token axis so the core's OWN 1024
tokens always occupy columns 0:1024 (softmax is invariant to key order), which
lets one compiled program serve all 8 cores with z_q a plain slice of z_full.

Layout: activations are feature-major ([feature, token]).  Weights are
host-pre-tiled so every DMA is one contiguous block; LN affines are folded into
downstream weights; LN stats come from ones-vector matmuls.

Attention is restructured around the cost model:
  * scores st[key, query] (f32r QK, 512-wide moving) -> exp on ACT over a
    2-head [128, 1024] PSUM tile -> AV with pexp as the STATIONARY operand:
    out[query, dv] = sum_k pexp[k, q] v[k, d].  This halves tensor-engine AV
    time vs the [dv+1, q] orientation (moving dim 65 vs 512 per key tile)
    and gives token-major AV output that is transposed back per 128x128
    block on the PE (free: Ldweights costs nothing, transpose 1cyc/row bf16).
  * the softmax denominator rides as an appended ones column of V (col 64),
    the per-query shift M rides as contraction row 64 (ones in kaug, -M in qt).
  * queries are processed in two 512-column segments over all 12 heads; the
    out-projection + LN2 + FFN of segment 0 overlaps segment 1's ACT-bound
    exp work, keeping the PE busy.  K (f32r kaug) is bounced through DRAM
    between segments instead of recomputed, as (lo, hi) half-sequence tiles
    so the reload cycles buffers at half-sequence granularity.
  * QK runs one t-step ahead of the exp stream so ACT rolls exp-to-exp.

FFN runs on fp8 DoubleRow matmuls (cost-model rate 0.5 cyc per output
column, 256-deep contraction): weights are dual-fp8 (w*64 = wa + wb, both
e4m3, residual-encoded; /64 folded into the gelu scale / bias adds).  FFN1
is three-term (za@wa + zb@wa + za@wb with z2 itself dual-fp8 from the LN2
writer, cross term dropped) at 0.75x bf16 cost; FFN2 contracts single-fp8
h1 (gelu writes e4m3 directly) against dual w2 at 0.5x.  Measured end-to-end
max rel err ~1.0e-2 vs the f32 reference (gate 2e-2).

Precision: score path (z, wq/wk, q, k, QK) in float32r; V/out bf16;
FFN dual-fp8 as above; x residual staged bf16.
"""

import sys

sys.path.insert(0, "/opt/trn_rl_repo")

import numpy as np

import concourse.bacc as bacc
import concourse.bass as bass
import concourse.tile as tile
from concourse import mybir
from concourse.bass_utils import run_bass_kernel_spmd

F32 = mybir.dt.float32
F32R = mybir.dt.float32r
BF16 = mybir.dt.bfloat16
FP8 = mybir.dt.float8e4
AF = mybir.ActivationFunctionType
ALU = mybir.AluOpType
DR = mybir.MatmulPerfMode.DoubleRow
W8S = 64.0  # fp8 weight pre-scale (keeps w out of the subnormal range)
RCP_W8S = 1.0 / W8S

D = 768
H = 12
HP = 6  # head pairs
DH = 64
F = 3072
B = 4
N = 2048
NQ = 1024  # tokens owned per core
P = 128
KT = D // P  # 6 feature k-tiles
MT = F // P  # 24 mlp-hidden tiles
NKT = N // P  # 16 key-token tiles
SG = 512  # query segment width
NSEG = NQ // SG  # 2
QB = SG // P  # 4 query blocks per segment
SCALE = float(DH**0.5)  # reference MULTIPLIES scores by sqrt(dh)
EXP_BIAS = -40.0  # pad on the exp argument (post-scale logit units)
SSTRIDE = 16  # key sampling stride for the shift estimate
NS = N // SSTRIDE
EPS = 1e-5
CK = 512


def build_nc():
    nc = bacc.Bacc("TRN2", target_bir_lowering=False, debug=False)

    xT = nc.dram_tensor("xT", [N // CK, P, KT, CK], F32R, kind="ExternalInput")
    xTq2 = nc.dram_tensor("xTq2", [D, NQ], BF16, kind="ExternalInput")
    wq = nc.dram_tensor("wq", [KT, P, KT, P], F32R, kind="ExternalInput")
    wk = nc.dram_tensor("wk", [KT, P, KT, P], F32R, kind="ExternalInput")
    wv = nc.dram_tensor("wv", [2, P, KT, CK], F32R, kind="ExternalInput")
    wo = nc.dram_tensor("wo", [KT, P, KT, P], BF16, kind="ExternalInput")
    # FFN weights: dual-fp8 DoubleRow tiles, inner dim order (kp, ab, dr)
    w1 = nc.dram_tensor("w1", [MT, P, 4 * (KT // 2), P], FP8, kind="ExternalInput")
    w2 = nc.dram_tensor("w2", [KT, P, 4 * (MT // 2), P], FP8, kind="ExternalInput")
    bq = nc.dram_tensor("bq", [D], F32, kind="ExternalInput")
    bv = nc.dram_tensor("bv", [D], F32R, kind="ExternalInput")
    bo = nc.dram_tensor("bo", [D], F32, kind="ExternalInput")
    b1 = nc.dram_tensor("b1", [F], F32, kind="ExternalInput")
    b2 = nc.dram_tensor("b2", [D], F32, kind="ExternalInput")
    ident = nc.dram_tensor("ident", [P, P], BF16, kind="ExternalInput")
    onesd = nc.dram_tensor("onesd", [1, N], F32R, kind="ExternalInput")
    yT = nc.dram_tensor("yT", [D, NQ], F32, kind="ExternalOutput")
    mscratch = nc.dram_tensor("mscratch", [H, NQ], F32R)
    qdram = nc.dram_tensor("qdram", [H, DH, SG], F32R)
    kdram = nc.dram_tensor("kdram", [H, DH + 1, N], F32R)

    with tile.TileContext(nc) as tc:
        _body(tc, xT, xTq2, wq, wk, wv, wo, w1, w2, bq, bv, bo, b1, b2,
              ident, onesd, yT, mscratch, qdram, kdram)
    nc.compile()
    return nc


class Ctx:
    pass


def _layernorm_fm(tc, g, load_fn, ncols, name, zpool, out_dt, wp, rstd_on_act=False,
                  lnps=None, on_chunk=None, z_tiles=None, rb=1, fp8_pair=None,
                  xsq_dve=False, m2f_dve=False):
    """Feature-major layernorm (affine folded into downstream weights).

    load_fn(k, c, sl) -> AP of a [128, CK] chunk of the input.
    Returns KT tiles [128, ncols] of dtype out_dt holding z = (x - mu) * rstd.
    With fp8_pair=(za, zb) (paired [128, 2, ncols] fp8 tiles) the result is
    written as a dual-fp8 pair instead: za = fp8(z), zb = fp8(z - za).
    """
    nc = tc.nc
    nch = ncols // CK
    ones_row = g.ones_row_r if out_dt == F32R else g.ones_row_b
    row_dt = F32R if out_dt == F32R else BF16

    if fp8_pair is not None:
        z_sb = None
    else:
        z_sb = z_tiles if z_tiles is not None else [
            zpool.tile([P, ncols], out_dt, name=f"{name}_z{k}") for k in range(KT)]
    for c in range(nch):
        sl = slice(c * CK, (c + 1) * CK)
        if lnps is not None:
            ps = lnps.tile([33, CK], F32, tag="lnst", bufs=2, name="lnst_ps")
        else:
            ps = g.psum_mm.tile([33, CK], F32, tag="mm", name="ln_ps")
        p1, p2 = ps[0:1, :], ps[32:33, :]
        for k in range(KT):
            xc = load_fn(k, c, sl)
            if xc.dtype == F32R:
                nc.tensor.matmul(p1[:], g.ones_col_r[:], xc,
                                 start=(k == 0), stop=(k == KT - 1))
            elif xc.dtype == BF16:
                nc.tensor.matmul(p1[:], g.ones_col[:], xc,
                                 start=(k == 0), stop=(k == KT - 1))
            else:
                xb = wp.tile([P, CK], BF16, tag="ln_xb")
                nc.vector.tensor_copy(out=xb[:], in_=xc)
                nc.tensor.matmul(p1[:], g.ones_col[:], xb[:],
                                 start=(k == 0), stop=(k == KT - 1))
            xsq = wp.tile([P, CK], BF16, tag="ln_xsq", bufs=2)
            if xsq_dve:
                nc.vector.tensor_mul(out=xsq[:], in0=xc, in1=xc)
            else:
                nc.scalar.activation(out=xsq[:], in_=xc, func=AF.Square)
            nc.tensor.matmul(p2[:], g.ones_col[:], xsq[:], start=(k == 0), stop=(k == KT - 1))
        s1 = wp.tile([1, CK], F32, name="s1r", tag="lnr_a", bufs=rb)
        s2 = wp.tile([1, CK], F32, name="s2r", tag="lnr_b", bufs=rb)
        rt = wp.tile([1, CK], F32, name="rtr", tag="lnr_c", bufs=1)
        y = wp.tile([1, CK], F32, name="yr", tag="lnr_y", bufs=1)
        nc.vector.tensor_scalar_mul(out=s1[:], in0=p1[:], scalar1=1.0 / D)
        nc.vector.tensor_scalar(out=s2[:], in0=p2[:], scalar1=1.0 / D,
                                scalar2=EPS, op0=ALU.mult, op1=ALU.add)
        nc.vector.tensor_mul(out=rt[:], in0=s1[:], in1=s1[:])  # mu^2
        nc.vector.tensor_sub(out=s2[:], in0=s2[:], in1=rt[:])  # var + eps
        if rstd_on_act:
            # rstd = exp(-0.5*ln(var)): fine where ACT is idle (preamble);
            # costs two act-table loads per chunk
            nc.scalar.activation(out=rt[:], in_=s2[:], func=AF.Ln)
            nc.scalar.activation(out=y[:], in_=rt[:], func=AF.Exp, scale=-0.5)
        else:
            # rstd = rsqrt(var) via Newton on DVE (vars are ~1, so a linear
            # seed converges in 3 steps); keeps LN2 off the ACT tables while
            # the attention exp stream runs
            nc.vector.tensor_scalar(out=y[:], in0=s2[:], scalar1=-0.5,
                                    scalar2=1.5, op0=ALU.mult, op1=ALU.add)
            for _ in range(3):
                nc.vector.tensor_mul(out=rt[:], in0=y[:], in1=y[:])  # y^2
                nc.vector.tensor_mul(out=rt[:], in0=rt[:], in1=s2[:])  # v*y^2
                nc.vector.tensor_scalar(out=rt[:], in0=rt[:], scalar1=-0.5,
                                        scalar2=1.5, op0=ALU.mult, op1=ALU.add)
                nc.vector.tensor_mul(out=y[:], in0=y[:], in1=rt[:])
        nc.vector.tensor_mul(out=s1[:], in0=s1[:], in1=y[:])  # m2 = mu*rstd
        rstd_r = wp.tile([1, CK], row_dt, name="rstdr", tag="lnr_d", bufs=2)
        nc.vector.tensor_copy(out=rstd_r[:], in_=y[:])
        m2_r = wp.tile([1, CK], row_dt, name="m2r", tag="lnr_e", bufs=2)
        nc.vector.tensor_copy(out=m2_r[:], in_=s1[:])

        if lnps is not None:
            rstdF = lnps.tile([P, CK], F32, tag="lnbc", bufs=4, name="rstdF")
            m2F = lnps.tile([P, CK], F32, tag="lnbc", bufs=4, name="m2F")
        else:
            rstdF = g.psum_mm.tile([P, CK], F32, tag="mm")
            m2F = g.psum_mm.tile([P, CK], F32, tag="mm")
        nc.tensor.matmul(rstdF[:], ones_row[:], rstd_r[:], start=True, stop=True)
        nc.tensor.matmul(m2F[:], ones_row[:], m2_r[:], start=True, stop=True)
        m2F_sb = wp.tile([P, CK], F32, tag="m2fsb", bufs=2)
        if m2f_dve:
            nc.vector.tensor_copy(out=m2F_sb[:], in_=m2F[:])
        else:
            nc.scalar.copy(out=m2F_sb[:], in_=m2F[:])
        for k in range(KT):
            xc = load_fn(k, c, sl)
            tmp = wp.tile([P, CK], F32, tag="lntmp", bufs=2)
            nc.vector.tensor_mul(out=tmp[:], in0=xc, in1=rstdF[:])
            if fp8_pair is None:
                nc.gpsimd.tensor_sub(out=z_sb[k][:, sl], in0=tmp[:], in1=m2F_sb[:])
            else:
                za, zb = fp8_pair
                a_ap = za[k // 2][:, k % 2, sl]
                nc.vector.tensor_sub(out=tmp[:], in0=tmp[:], in1=m2F_sb[:])
                nc.gpsimd.tensor_copy(out=a_ap, in_=tmp[:])
                nc.gpsimd.tensor_sub(out=zb[k // 2][:, k % 2, sl],
                                     in0=tmp[:], in1=a_ap)
        if on_chunk is not None:
            on_chunk(c)
    return z_sb


def _body(tc, xT, xTq2, wq, wk, wv, wo, w1, w2, bq, bv, bo, b1, b2,
          ident, onesd, yT, mscratch, qdram, kdram):
    nc = tc.nc
    from contextlib import ExitStack

    with ExitStack() as es:
        g = Ctx()
        g.singles = es.enter_context(tc.tile_pool(name="singles", bufs=1))
        g.rows = es.enter_context(tc.tile_pool(name="rows", bufs=1))
        g.work = es.enter_context(tc.tile_pool(name="work", bufs=2))
        g.wpool = es.enter_context(tc.tile_pool(name="wpool", bufs=2))
        # PSUM: mm [128,512] x2 up front; st/av created after the LN phase
        # (LN1 borrows their banks for chunk pipelining)
        g.psum_mm = es.enter_context(tc.tile_pool(name="psum_mm", bufs=2, space="PSUM"))

        x_pre = {}

        g.ones_col = g.singles.tile([P, 1], BF16, name="ones_col")
        nc.vector.memset(g.ones_col[:], 1.0)
        g.ones_col_f = g.singles.tile([P, 1], F32, name="ones_col_f")
        nc.vector.memset(g.ones_col_f[:], 1.0)
        g.ones_col_r = g.singles.tile([P, 1], F32R, name="ones_col_r")
        nc.vector.tensor_copy(out=g.ones_col_r[:], in_=g.ones_col_f[:])
        g.ones_row_b = g.singles.tile([1, P], BF16, name="ones_row_b")
        nc.vector.memset(g.ones_row_b[:], 1.0)
        g.ones_row_f = g.singles.tile([1, P], F32, name="ones_row_f")
        nc.vector.memset(g.ones_row_f[:], 1.0)
        g.ones_row_r = g.singles.tile([1, P], F32R, name="ones_row_r")
        nc.vector.tensor_copy(out=g.ones_row_r[:], in_=g.ones_row_f[:])
        g.eps_sb = g.singles.tile([1, 1], F32, name="eps")
        nc.vector.memset(g.eps_sb[:], EPS)
        g.expb_sb = g.singles.tile([P, 1], F32, name="expb")
        nc.vector.memset(g.expb_sb[:], EXP_BIAS)
        g.zeros_row = g.singles.tile([1, QB * (DH + 1)], BF16, name="zeros_row")
        nc.vector.memset(g.zeros_row[:], 0.0)
        g.gelu_gate = g.singles.tile([P, 1], F32, name="gelu_gate")
        g.ident = g.singles.tile([P, P], BF16, name="ident")

        def load_bias_cols(dram, n, name):
            t = g.singles.tile([P, n // P], F32, name=name)
            return t, lambda: nc.scalar.dma_start(
                out=t[:], in_=dram.ap().rearrange("(j p) -> p j", p=P))

        bo_sb, ld_bo = load_bias_cols(bo, D, "bo_sb")
        b1_sb, ld_b1 = load_bias_cols(b1, F, "b1_sb")
        b2_sb, ld_b2 = load_bias_cols(b2, D, "b2_sb")
        bq_sb, ld_bq = load_bias_cols(bq, D, "bq_sb")
        bv_row = g.singles.tile([1, D], F32R, name="bv_row")

        def emit_const_dmas():
            # emitted only after the first two x-chunk DMAs: the (shared)
            # DMA issue queue drains in emission order and the x stream is
            # the LN1 critical path.  These ride the ACT hwdge queue.
            nc.scalar.dma_start(out=g.ident[:], in_=ident.ap())
            nc.scalar.dma_start(out=bv_row[:],
                                in_=bv.ap().rearrange("(a n) -> a n", a=1))
            for ld in (ld_bq, ld_b1, ld_bo, ld_b2):
                ld()

        def stream_loader(dram, pool):
            state = {}
            def load(k, c, sl):
                if state.get("c") != c:
                    if c in x_pre:
                        state["t"] = x_pre[c]
                    else:
                        t = pool.tile([P, KT, CK], F32R, tag="xstream", name="xs")
                        nc.sync.dma_start(out=t[:], in_=dram.ap()[c])
                        state["t"] = t
                    state["c"] = c
                return state["t"][:, k, :]
            return load

        # ---------- persistent activation tiles ----------
        vpool = es.enter_context(tc.tile_pool(name="vpool", bufs=1))
        v_sb = [vpool.tile([P, H, DH + 1], BF16, name=f"v{t}") for t in range(NKT)]
        opool = es.enter_context(tc.tile_pool(name="opool", bufs=1))
        o_sb = [[opool.tile([P, SG], BF16, name=f"o0_{j}") for j in range(KT)], None]
        xmid = [None, None]  # filled per segment from scoped pools

        for t in range(NKT):
            nc.gpsimd.memset(v_sb[t][:], 1.0)  # col 64 of each head stays 1.0

        def w_load(dram, j, tag, dt=BF16, nk=KT, w=P, pool=None, bufs=None):
            t = (pool or g.wpool).tile([P, nk, w], dt, tag=tag, name=f"wt_{tag}{j}",
                                       **({"bufs": bufs} if bufs else {}))
            nc.sync.dma_start(out=t[:], in_=dram.ap()[j])
            return t


        def q_proj_pair(jp, z, cols, out_even, out_odd, spool, on_act=False):
            """Project the head pair jp's queries for z[:, cols].

            Both heads come out of one [128, SG] psum (full PE width).  The
            odd head's rows 64:128 are biased into a staging tile and DMA'd
            to out_odd (partition shift needs a DMA).  With on_act the bias
            adds run as ACT Identity-with-bias (keeps the preamble DVE queue
            clear) and the even head is written directly to its SBUF AP."""
            wqb = w_load(wq, jp, "wqk", dt=F32R, pool=wqkp)
            pt = g.psum_mm.tile([P, SG], F32, tag="mm")
            for k in range(KT):
                nc.tensor.matmul(pt[:], wqb[:, k, :], z[k][:, cols],
                                 start=(k == 0), stop=(k == KT - 1))
            stg = spool.tile([P, SG], F32R, tag="qstg")
            if on_act:
                nc.scalar.activation(out=out_even, in_=pt[0:DH, :],
                                     func=AF.Identity,
                                     bias=bq_sb[0:DH, jp : jp + 1], scale=1.0)
                nc.scalar.activation(out=stg[DH:P, :], in_=pt[DH:P, :],
                                     func=AF.Identity,
                                     bias=bq_sb[DH:P, jp : jp + 1], scale=1.0)
            else:
                nc.vector.tensor_scalar_add(out=stg[0:DH, :], in0=pt[0:DH, :],
                                            scalar1=bq_sb[0:DH, jp : jp + 1])
                nc.vector.tensor_scalar_add(out=stg[DH:P, :], in0=pt[DH:P, :],
                                            scalar1=bq_sb[DH:P, jp : jp + 1])
                nc.sync.dma_start(out=out_even, in_=stg[0:DH, :])
            nc.sync.dma_start(out=out_odd, in_=stg[DH:P, :])

        def m_shift(h, q_sb, seg):
            """Sampled row-max shift for head h, queries of segment seg.

            q_sb rows 0:64 hold the biased q.  Writes -max to mscratch[h, seg].
            kaugs[h] is an (lo, hi) pair of [DH+1, N/2] tiles; samples come
            half from each (one accumulation group, disjoint psum regions)."""
            m_sb = g.work.tile([P, QB], F32R, tag="msb")
            for qt_i in range(QB):
                ss = g.psum_mm.tile([P, CK], F32, tag="mm")
                for half in range(2):
                    ksamp = (kaugs[h][half][0:DH, :]
                             .rearrange("p (n t) -> p n t", t=SSTRIDE)[:, :, 0:1])
                    nc.tensor.matmul(ss[:, half * (NS // 2) : (half + 1) * (NS // 2)],
                                     q_sb[0:DH, qt_i * P : (qt_i + 1) * P],
                                     ksamp, start=(half == 0), stop=(half == 1),
                                     skip_group_check=True)
                nc.vector.tensor_reduce(
                    out=m_sb[:, qt_i : qt_i + 1], in_=ss[:, :NS],
                    axis=mybir.AxisListType.X, op=ALU.max, negate=True,
                )
            nc.sync.dma_start(
                out=mscratch.ap()[h : h + 1, seg * SG : (seg + 1) * SG]
                    .rearrange("a (c p) -> a p c", p=P),
                in_=m_sb[:],
            )

        def k_fills(jp, kpool):
            """Fine-grained filler closures for pair jp's K projection:
            [setup+chunk0, chunk1, chunk2, chunk3+kdram, shift-rows].

            kaug is stored as (lo, hi) [DH+1, N/2] tiles so the seg-1
            reload can cycle buffers at half-sequence granularity.  One
            [128, CK] psum per chunk covers both heads (full PE width); the
            odd head's rows 64:128 bounce through a staging tile + DMA."""
            NH = N // 2
            st = {}

            def chunk(c):
                ks, wkb = st["ks"], st["wkb"]
                half, co = c // 2, (c % 2) * CK
                osl = slice(co, co + CK)
                sl = slice(c * CK, (c + 1) * CK)
                pt = g.psum_mm.tile([P, CK], F32, tag="mm")
                for k in range(KT):
                    nc.tensor.matmul(pt[:], wkb[:, k, :], z_full[k][:, sl],
                                     start=(k == 0), stop=(k == KT - 1))
                nc.vector.tensor_copy(out=ks[0][half][0:DH, osl], in_=pt[0:DH, :])
                stg = qspool.tile([P, CK], F32R, tag="kstg", bufs=2)
                nc.vector.tensor_copy(out=stg[DH:P, :], in_=pt[DH:P, :])
                nc.sync.dma_start(out=ks[1][half][0:DH, osl], in_=stg[DH:P, :])

            def setup():
                ks = []
                for s in range(2):
                    h = 2 * jp + s
                    pair = []
                    for half in range(2):
                        kaug = kpool.tile([DH + 1, NH], F32R,
                                          name=f"kaug{h}_{half}", tag="kaug")
                        nc.sync.dma_start(out=kaug[DH : DH + 1, :],
                                          in_=onesd.ap()[0:1, 0:NH])
                        pair.append(kaug)
                    ks.append(tuple(pair))
                st["ks"] = ks
                st["wkb"] = w_load(wk, jp, "wqk", dt=F32R, pool=wqkp)
                chunk(0)

            def finish():
                chunk(3)
                for s in range(2):
                    for half in range(2):
                        nc.sync.dma_start(
                            out=kdram.ap()[2 * jp + s][:, half * NH : (half + 1) * NH],
                            in_=st["ks"][s][half][:])

            def shift():
                for s in range(2):
                    kaugs[2 * jp + s] = st["ks"][s]
                m_rows(jp)

            return [setup, lambda: chunk(1), lambda: chunk(2), finish, shift]

        def attn_pair(jp, seg, fillers, pexpool):
            """Attention for head pair jp over segment seg's 512 queries.

            fillers: list of zero-arg callables emitting independent PE work,
            interleaved into the t-loop to cover ACT-bound stretches."""
            h0, h1 = 2 * jp, 2 * jp + 1
            qts = (qt0[h0], qt0[h1]) if seg == 0 else (qt1s[h0], qt1s[h1])
            av = [g.psum_av.tile([P, QB * (DH + 1)], F32, tag="av", name=f"av{s}")
                  for s in range(2)]
            # The 4 query-block accumulation regions share one PSUM zero
            # region (2KB bank), so start_tensor_calc must fire exactly once
            # per bank: zero the whole tile with one K=1 matmul, then
            # accumulate with start=False.
            for s in range(2):
                nc.tensor.matmul(av[s][:], g.ones_row_b[:], g.zeros_row[:],
                                 start=True, stop=True)
            nfill = len(fillers)
            fi = 0

            def qk(t):
                st = g.psum_st.tile([P, 2 * SG], F32, tag="st")
                for s in range(2):
                    kh = kaugs[2 * jp + s][t // 8]
                    nc.tensor.matmul(st[:, s * SG : (s + 1) * SG],
                                     kh[:, (t % 8) * P : (t % 8 + 1) * P],
                                     qts[s][:], start=True, stop=True)
                return st

            # QK runs one step ahead of the exp stream: st(t+1) is issued
            # right after exp(t) so ACT rolls exp-to-exp without waiting on
            # PE, and fillers drain in the AV/exp shadow.
            st = qk(0)
            for t in range(NKT):
                pexp = pexpool.tile([P, 2 * SG], BF16, tag="pexp")
                nc.scalar.activation(out=pexp[:], in_=st[:], func=AF.Exp,
                                     scale=SCALE, bias=g.expb_sb[:])
                if t + 1 < NKT:
                    st = qk(t + 1)
                # interleave filler work so the PE queue stays fed while
                # exp(t) is still on ACT
                while fi * NKT < (t + 1) * nfill:
                    fillers[fi]()
                    fi += 1
                for s in range(2):
                    h = 2 * jp + s
                    for qb in range(QB):
                        nc.tensor.matmul(
                            av[s][:, qb * (DH + 1) : (qb + 1) * (DH + 1)],
                            pexp[:, s * SG + qb * P : s * SG + (qb + 1) * P],
                            v_sb[t][:, h, :],
                            start=False, stop=(t == NKT - 1),
                            skip_group_check=True)
            # normalize (token-major), then transpose pairs back to
            # feature-major o_sb via PE
            otok = g.work.tile([P, QB, P], BF16, tag="otok")
            rr = [g.work.tile([P, QB, 1], F32, tag="attn_r", bufs=4, name=f"r{s}")
                  for s in range(2)]
            for s in range(2):
                nc.vector.reciprocal(
                    out=rr[s][:],
                    in_=av[s][:].rearrange("p (q c) -> p q c", c=DH + 1)[:, :, DH : DH + 1])
            for qb in range(QB):
                for s in range(2):
                    nc.vector.tensor_scalar_mul(
                        out=otok[:, qb, s * DH : (s + 1) * DH],
                        in0=av[s][:, qb * (DH + 1) : qb * (DH + 1) + DH],
                        scalar1=rr[s][:, qb, :])
            for qb in range(QB):
                ptr = g.psum_av.tile([P, P], BF16, tag="av", name="ptr")
                nc.tensor.transpose(ptr[:], otok[:, qb, :], g.ident[:])
                nc.vector.tensor_copy(out=o_sb[seg][jp][:, qb * P : (qb + 1) * P],
                                      in_=ptr[:])

        def out_proj_fills(seg, wfp):
            """Closures: o_sb[seg] @ wo + bo + x residual -> xmid[seg]."""
            xq2 = [wfp.tile([P, SG], BF16, tag="xq2", bufs=6, name=f"xq2_{seg}_{k}")
                   for k in range(KT)]

            def xq2_load():
                for k in range(KT):
                    nc.sync.dma_start(
                        out=xq2[k][:],
                        in_=xTq2.ap()[k * P : (k + 1) * P, seg * SG : (seg + 1) * SG])

            def op_j(j):
                pt = g.psum_mm.tile([P, SG], F32, tag="mm")
                for k in range(KT):
                    nc.tensor.matmul(pt[:], wobs[j][:, k, :], o_sb[seg][k][:],
                                     start=(k == 0), stop=(k == KT - 1))
                tmp = wfp.tile([P, SG], F32, tag="tmpf4")
                nc.vector.tensor_scalar_add(out=tmp[:], in0=pt[:],
                                            scalar1=bo_sb[:, j : j + 1])
                nc.vector.tensor_add(out=xmid[seg][j][:], in0=tmp[:], in1=xq2[j][:])

            return [xq2_load] + [lambda j=j: op_j(j) for j in range(KT)]

        def ffn_fills(seg, z2pool, h1pool, wfp, defer_gelu, rstd_on_act=False):
            """Closure groups for LN2 + FFN over segment seg's tokens.

            FFN1 runs as dual-fp8 DoubleRow (three-term: za@wa + zb@wa +
            za@wb, cross term dropped); FFN2 as single-fp8 h1 against dual
            fp8 w2.  Weights carry a x64 pre-scale, corrected in the gelu
            scale / bias adds.  With defer_gelu, ffn1 stores biased pre-gelu
            h1 via DVE (so no Gelu touches ACT while the attention exp
            stream is running) and the returned gelu fills apply Gelu
            later, writing the fp8 h1 tiles.  Returns
            (pre_fills, gelu_fills, ffn2_fills)."""
            KP = KT // 2
            MP = MT // 2
            z2a = [z2pool.tile([P, 2, SG], FP8, name=f"z2a_{seg}_{kp}")
                   for kp in range(KP)]
            z2b = [z2pool.tile([P, 2, SG], FP8, name=f"z2b_{seg}_{kp}")
                   for kp in range(KP)]
            h1q = [h1pool.tile([P, 2, SG], FP8, name=f"h1_{seg}_{mp}")
                   for mp in range(MP)]
            h1pre = ([h1pool.tile([P, SG], BF16, name=f"h1p_{seg}_{m}")
                      for m in range(MT)] if defer_gelu else None)

            # LN2 as fine-grained closures (so the seg-1 filler interleave
            # spreads its DVE/PE load instead of spiking).  xsq + m2F stay
            # off ACT: LN2(0) runs inside the ACT-bound exp stream, LN2(1)
            # runs while ACT drains the gelu backlog.
            lns = {}

            def ln_stat(k):
                if k == 0:
                    lns["ps"] = g.psum_mm.tile([33, SG], F32, tag="mm",
                                               name="ln2_ps")
                ps = lns["ps"]
                xc = xmid[seg][k][:]
                nc.tensor.matmul(ps[0:1, :], g.ones_col[:], xc,
                                 start=(k == 0), stop=(k == KT - 1))
                xsq = wfp.tile([P, SG], BF16, tag="ln_xsq", bufs=2)
                nc.vector.tensor_mul(out=xsq[:], in0=xc, in1=xc)
                nc.tensor.matmul(ps[32:33, :], g.ones_col[:], xsq[:],
                                 start=(k == 0), stop=(k == KT - 1))

            def ln_rows():
                ps = lns["ps"]
                s1 = wfp.tile([1, SG], F32, name="s1r", tag="lnr_a")
                s2 = wfp.tile([1, SG], F32, name="s2r", tag="lnr_b")
                rt = wfp.tile([1, SG], F32, name="rtr", tag="lnr_c", bufs=1)
                y = wfp.tile([1, SG], F32, name="yr", tag="lnr_y", bufs=1)
                nc.vector.tensor_scalar_mul(out=s1[:], in0=ps[0:1, :],
                                            scalar1=1.0 / D)
                nc.vector.tensor_scalar(out=s2[:], in0=ps[32:33, :],
                                        scalar1=1.0 / D, scalar2=EPS,
                                        op0=ALU.mult, op1=ALU.add)
                nc.vector.tensor_mul(out=rt[:], in0=s1[:], in1=s1[:])
                nc.vector.tensor_sub(out=s2[:], in0=s2[:], in1=rt[:])
                if rstd_on_act:
                    nc.scalar.activation(out=rt[:], in_=s2[:], func=AF.Ln)
                    nc.scalar.activation(out=y[:], in_=rt[:], func=AF.Exp,
                                         scale=-0.5)
                else:
                    nc.vector.tensor_scalar(out=y[:], in0=s2[:], scalar1=-0.5,
                                            scalar2=1.5, op0=ALU.mult,
                                            op1=ALU.add)
                    for _ in range(3):
                        nc.vector.tensor_mul(out=rt[:], in0=y[:], in1=y[:])
                        nc.vector.tensor_mul(out=rt[:], in0=rt[:], in1=s2[:])
                        nc.vector.tensor_scalar(out=rt[:], in0=rt[:],
                                                scalar1=-0.5, scalar2=1.5,
                                                op0=ALU.mult, op1=ALU.add)
                        nc.vector.tensor_mul(out=y[:], in0=y[:], in1=rt[:])
                nc.vector.tensor_mul(out=s1[:], in0=s1[:], in1=y[:])
                rstd_r = wfp.tile([1, SG], BF16, name="rstdr", tag="lnr_d")
                nc.vector.tensor_copy(out=rstd_r[:], in_=y[:])
                m2_r = wfp.tile([1, SG], BF16, name="m2r", tag="lnr_e")
                nc.vector.tensor_copy(out=m2_r[:], in_=s1[:])
                lns["rstd_r"], lns["m2_r"] = rstd_r, m2_r

            def ln_bcast():
                rstdF = g.psum_mm.tile([P, SG], F32, tag="mm")
                m2F = g.psum_mm.tile([P, SG], F32, tag="mm")
                nc.tensor.matmul(rstdF[:], g.ones_row_b[:], lns["rstd_r"][:],
                                 start=True, stop=True)
                nc.tensor.matmul(m2F[:], g.ones_row_b[:], lns["m2_r"][:],
                                 start=True, stop=True)
                m2F_sb = wfp.tile([P, SG], F32, tag="m2fsb", bufs=2)
                nc.vector.tensor_copy(out=m2F_sb[:], in_=m2F[:])
                lns["rstdF"], lns["m2F_sb"] = rstdF, m2F_sb

            def ln_z(k):
                xc = xmid[seg][k][:]
                tmp = wfp.tile([P, SG], F32, tag="lntmp", bufs=2)
                nc.vector.tensor_mul(out=tmp[:], in0=xc, in1=lns["rstdF"][:])
                nc.vector.tensor_sub(out=tmp[:], in0=tmp[:], in1=lns["m2F_sb"][:])
                a_ap = z2a[k // 2][:, k % 2, :]
                nc.gpsimd.tensor_copy(out=a_ap, in_=tmp[:])
                nc.gpsimd.tensor_sub(out=z2b[k // 2][:, k % 2, :],
                                     in0=tmp[:], in1=a_ap)

            ln_fills = ([lambda k=k: ln_stat(k) for k in range(KT)]
                        + [ln_rows, ln_bcast]
                        + [lambda k=k: ln_z(k) for k in range(KT)])

            def ffn1(m):
                w1b = w_load(w1, m, "w1", dt=FP8, nk=4 * KP, pool=wfp, bufs=4)
                pt = g.psum_mm.tile([P, SG], F32, tag="mm")
                terms = []
                for kp in range(KP):
                    wa = w1b[:, 4 * kp : 4 * kp + 2, :]
                    wb = w1b[:, 4 * kp + 2 : 4 * kp + 4, :]
                    terms += [(wa, z2a[kp]), (wa, z2b[kp]), (wb, z2a[kp])]
                for i, (wv, zv) in enumerate(terms):
                    nc.tensor.matmul(pt[:], wv, zv[:], start=(i == 0),
                                     stop=(i == len(terms) - 1), perf_mode=DR)
                if defer_gelu:
                    nc.vector.tensor_scalar(out=h1pre[m][:], in0=pt[:],
                                            scalar1=RCP_W8S,
                                            scalar2=b1_sb[:, m : m + 1],
                                            op0=ALU.mult, op1=ALU.add)
                else:
                    nc.scalar.activation(out=h1q[m // 2][:, m % 2, :], in_=pt[:],
                                         func=AF.Gelu, bias=b1_sb[:, m : m + 1],
                                         scale=RCP_W8S)

            def gelu(m):
                # bias is numerically zero but reads the gate tile, which is
                # written only after the last attention output lands: all
                # gelus become ready together, after the exp stream, so the
                # act-table pass switches tables once instead of thrashing
                nc.scalar.activation(out=h1q[m // 2][:, m % 2, :],
                                     in_=h1pre[m][:], func=AF.Gelu,
                                     bias=g.gelu_gate[:], scale=1.0)

            def ffn2(j):
                w2b = w_load(w2, j, "w2", dt=FP8, nk=4 * MP, pool=wfp, bufs=3)
                pt = g.psum_mm.tile([P, SG], F32, tag="mm")
                for i in range(2 * MP):
                    mp, ab = i // 2, i % 2
                    base = 4 * mp + 2 * ab
                    nc.tensor.matmul(pt[:], w2b[:, base : base + 2, :],
                                     h1q[mp][:], start=(i == 0),
                                     stop=(i == 2 * MP - 1), perf_mode=DR)
                tmp = wfp.tile([P, SG], F32, tag="tmpf")
                nc.vector.tensor_scalar(out=tmp[:], in0=pt[:], scalar1=RCP_W8S,
                                        scalar2=b2_sb[:, j : j + 1],
                                        op0=ALU.mult, op1=ALU.add)
                out_t = wfp.tile([P, SG], F32, tag="f2_out")
                nc.vector.tensor_add(out=out_t[:], in0=tmp[:], in1=xmid[seg][j][:])
                nc.sync.dma_start(
                    out=yT.ap()[j * P : (j + 1) * P, seg * SG : (seg + 1) * SG],
                    in_=out_t[:])

            pre = ln_fills + [lambda m=m: ffn1(m) for m in range(MT)]
            gelus = [] if not defer_gelu else [lambda m=m: gelu(m) for m in range(MT)]
            return pre, gelus, [lambda j=j: ffn2(j) for j in range(KT)]

        # ================= preamble + segment 0 =================
        kaugs = {}
        with ExitStack() as pre_es:
            zpool = pre_es.enter_context(tc.tile_pool(name="z", bufs=1))
            z_full = [zpool.tile([P, N], F32R, name=f"ln1_z{k}") for k in range(KT)]
            qpool0 = pre_es.enter_context(tc.tile_pool(name="qpool0", bufs=1))
            qt0 = [qpool0.tile([DH + 1, SG], F32R, name=f"qt0_{h}") for h in range(H)]
            wqkp = pre_es.enter_context(tc.tile_pool(name="wqkp", bufs=2))
            qspool = pre_es.enter_context(tc.tile_pool(name="qspool", bufs=2))
            vwp = pre_es.enter_context(tc.tile_pool(name="vwp", bufs=1))
            kp0 = pre_es.enter_context(tc.tile_pool(name="kp0", bufs=4))

            vwb_cache = {}

            def v_proj(ci, half, ts, on_act):
                """V projection for a 256-wide dv quarter, key tiles ts.
                The bias rides as a K=1 ones-row matmul so the psum->SBUF
                move is a plain copy, placeable on ACT (preamble) or DVE."""
                HW = CK // 2
                lo = ci * CK + half * HW
                w = min(D, lo + HW) - lo
                if w <= 0:
                    return
                key = (ci, half)
                if key not in vwb_cache:
                    wvb = vwp.tile([P, KT, HW], F32R, tag="wv",
                                   name=f"wvb{ci}_{half}", bufs=2)
                    nc.sync.dma_start(
                        out=wvb[:], in_=wv.ap()[ci][:, :, half * HW : (half + 1) * HW])
                    vwb_cache[key] = wvb
                wvb = vwb_cache[key]
                h0 = lo // DH
                nh = w // DH
                for t in ts:
                    pt = g.psum_mm.tile([P, HW], F32, tag="mm", name="vpt")
                    for k in range(KT):
                        nc.tensor.matmul(pt[:, :w],
                                         z_full[k][:, t * P : (t + 1) * P],
                                         wvb[:, k, :w],
                                         start=(k == 0), stop=False)
                    nc.tensor.matmul(pt[:, :w], g.ones_row_r[:],
                                     bv_row[0:1, lo : lo + w],
                                     start=False, stop=True)
                    src = pt[:, :w].rearrange("p (h d) -> p h d", d=DH)
                    dst = v_sb[t][:, h0 : h0 + nh, 0:DH]
                    if on_act:
                        nc.scalar.copy(out=dst, in_=src)
                    else:
                        nc.vector.tensor_copy(out=dst, in_=src)

            # x chunks 0/1 are the first DMAs emitted (LN1 critical path);
            # everything else queues behind them
            ln_xsp_cm = tc.tile_pool(name="xsp", bufs=2)
            xsp = ln_xsp_cm.__enter__()
            for c in (0, 1):
                t = xsp.tile([P, KT, CK], F32R, tag="xstream", name="xs")
                nc.sync.dma_start(out=t[:], in_=xT.ap()[c])
                x_pre[c] = t
            emit_const_dmas()

            # pair-0 K runs inside the LN chunk callback so the first QK can
            # start right after the last z chunk lands
            NH = N // 2
            kaug_p0 = []
            for h in range(2):
                pair = []
                for half in range(2):
                    kaug = kp0.tile([DH + 1, NH], F32R,
                                    name=f"kaug_p0_{h}_{half}", tag="kaug")
                    nc.sync.dma_start(out=kaug[DH : DH + 1, :],
                                      in_=onesd.ap()[0:1, 0:NH])
                    pair.append(kaug)
                kaug_p0.append(tuple(pair))
                kaugs[h] = kaug_p0[h]
            wkb0 = w_load(wk, 0, "wqk", dt=F32R, pool=wqkp)

            def ln_chunk_cb(c):
                if c == 1:
                    # z chunk 0 is live: queries(seg0) for all heads
                    for jp in range(HP):
                        q_proj_pair(jp, z_full, slice(0, SG),
                                    qt0[2 * jp][0:DH, :], qt0[2 * jp + 1][0:DH, :],
                                    qspool, on_act=True)
                half, co = c // 2, (c % 2) * CK
                osl = slice(co, co + CK)
                sl = slice(c * CK, (c + 1) * CK)
                pt = g.psum_mm.tile([P, CK], F32, tag="mm", name="kpt")
                for k in range(KT):
                    nc.tensor.matmul(pt[:], wkb0[:, k, :], z_full[k][:, sl],
                                     start=(k == 0), stop=(k == KT - 1))
                nc.vector.tensor_copy(out=kaug_p0[0][half][0:DH, osl],
                                      in_=pt[0:DH, :])
                stg = qspool.tile([P, CK], F32R, tag="kstg", bufs=2)
                nc.vector.tensor_copy(out=stg[DH:P, :], in_=pt[DH:P, :])
                nc.sync.dma_start(out=kaug_p0[1][half][0:DH, osl],
                                  in_=stg[DH:P, :])
                v_proj(0, 0, range(4 * c, 4 * c + 4), True)  # heads 0-3

            with ExitStack() as ln_es:
                lnw1 = ln_es.enter_context(tc.tile_pool(name="lnw1", bufs=1))
                lnps = ln_es.enter_context(tc.tile_pool(name="lnps", bufs=1,
                                                        space="PSUM"))
                _layernorm_fm(tc, g, stream_loader(xT, xsp), N, "ln1",
                              zpool, F32R, wp=lnw1, rstd_on_act=True,
                              lnps=lnps, on_chunk=ln_chunk_cb, z_tiles=z_full,
                              rb=2)
            ln_xsp_cm.__exit__(None, None, None)
            g.psum_st = es.enter_context(
                tc.tile_pool(name="psum_st", bufs=2, space="PSUM"))
            g.psum_av = es.enter_context(
                tc.tile_pool(name="psum_av", bufs=2, space="PSUM"))

            for s in range(2):
                for half in range(2):
                    nc.sync.dma_start(
                        out=kdram.ap()[s][:, half * NH : (half + 1) * NH],
                        in_=kaug_p0[s][half][:])

            kpool_a = pre_es.enter_context(tc.tile_pool(name="kpool_a", bufs=8))
            pexp_a = pre_es.enter_context(tc.tile_pool(name="pexp_a", bufs=5))

            def m_rows(jpn):
                for h in (2 * jpn, 2 * jpn + 1):
                    m_shift(h, qt0[h], 0)
                    nc.sync.dma_start(out=qt0[h][DH : DH + 1, :],
                                      in_=mscratch.ap()[h : h + 1, 0:SG])

            m_rows(0)

            qt1s = {}

            def load_pair(jp):
                """Stream kaug + q back for pair jp, compute seg-1 shift."""
                NH = N // 2
                for s in range(2):
                    h = 2 * jp + s
                    pair = []
                    for half in range(2):
                        kaug = kpool_b.tile([DH + 1, NH], F32R,
                                            name=f"kaug1_{h}_{half}",
                                            tag="kaug")
                        nc.sync.dma_start(
                            out=kaug[:],
                            in_=kdram.ap()[h][:, half * NH : (half + 1) * NH])
                        pair.append(kaug)
                    kaugs[h] = tuple(pair)
                    qt = qpool1.tile([DH + 1, SG], F32R, tag="qt1")
                    nc.sync.dma_start(out=qt[0:DH, :], in_=qdram.ap()[h])
                    m_shift(h, qt, 1)
                    nc.sync.dma_start(out=qt[DH : DH + 1, :],
                                      in_=mscratch.ap()[h : h + 1, SG:NQ])
                    qt1s[h] = qt

            # ============== segment 0 ==============
            # Fine-grained fillers, biased toward late pairs so the exp
            # stream's ACT backlog can drain while PE still has work:
            #   jp0-4: K(jp+1) in 5 chunks; jp0-3: one V quarter-half in
            #   two 4-tile chunks (V(q) completes one pair before its
            #   consumer); jp4-5: the Q(seg1) projections.
            def q1_fill(jp):
                # Q(seg1) goes straight to DRAM from the psum staging
                # tile; its shift row is computed in segment 1.
                return lambda: q_proj_pair(
                    jp, z_full, slice(SG, NQ),
                    qdram.ap()[2 * jp], qdram.ap()[2 * jp + 1], qspool)

            def seg0_filler(jp):
                fills = []
                if jp + 1 < HP:
                    fills += k_fills(jp + 1, kpool_a)
                if jp < 4:
                    ci, half = (0, 1) if jp < 2 else (1, 0)
                    lo = (jp % 2) * 8
                    for o in (0, 4):
                        fills.append(lambda ci=ci, half=half, lo=lo + o:
                                     v_proj(ci, half, range(lo, lo + 4), False))
                if jp == 4:
                    fills += [q1_fill(0), q1_fill(1)]
                elif jp == 5:
                    fills += [q1_fill(jpq) for jpq in range(2, HP)]
                return fills

            for jp in range(HP):
                attn_pair(jp, 0, seg0_filler(jp), pexp_a)
        # z_full + qt0 + seg0 kaug/pexp pools die here

        # ============== segment 1 (+ seg-0 FFN as filler) ==============
        with ExitStack() as late_es:
            opool1 = late_es.enter_context(tc.tile_pool(name="opool1", bufs=1))
            o_sb[1] = [opool1.tile([P, SG], BF16, name=f"o1_{j}") for j in range(KT)]
            wop = late_es.enter_context(tc.tile_pool(name="wop", bufs=1))
            # wo loads ride a lazy closure so their 6 DMAs queue after the
            # boundary-critical kaug/qt reloads, not before
            wobs = []

            def wo_load():
                wobs.extend(w_load(wo, j, f"wo{j}", pool=wop) for j in range(KT))

            xmpool = late_es.enter_context(tc.tile_pool(name="xmpool", bufs=1))
            xmid[0] = [xmpool.tile([P, SG], BF16, name=f"xm0_{j}") for j in range(KT)]
            xmid[1] = [xmpool.tile([P, SG], BF16, name=f"xm1_{j}") for j in range(KT)]

            h1p0 = late_es.enter_context(tc.tile_pool(name="h1p0", bufs=1))
            wfp = late_es.enter_context(tc.tile_pool(name="wfp", bufs=2))
            with ExitStack() as s1_es:
                z2p0 = s1_es.enter_context(tc.tile_pool(name="z2p0", bufs=1))
                qpool1 = s1_es.enter_context(tc.tile_pool(name="qpool1", bufs=3))
                kpool_b = s1_es.enter_context(tc.tile_pool(name="kpool_b", bufs=5))
                pexp_b = s1_es.enter_context(tc.tile_pool(name="pexp_b", bufs=3))

                load_pair(0)
                wo_load()

                # fillers: prefetches, then out-proj(seg0) + LN2(seg0) + the
                # FFN1(seg0) matmuls (gelu deferred to the tail so ACT stays
                # on the Exp table throughout the attention stream)
                ffn0_pre, ffn0_gelu, ffn0_2 = ffn_fills(0, z2p0, h1p0, wfp, True)
                fills_all = out_proj_fills(0, wfp) + ffn0_pre
                per = (len(fills_all) + HP - 1) // HP
                for jp in range(HP):
                    fills = []
                    if jp + 1 < HP:
                        fills.append(lambda jpn=jp + 1: load_pair(jpn))
                    fills += fills_all[jp * per : (jp + 1) * per]
                    attn_pair(jp, 1, fills, pexp_b)

            # ====== tail: gelu(seg0) + FFN2(seg0) + out-proj/FFN(seg1) ======
            with ExitStack() as s2_es:
                z2p1 = s2_es.enter_context(tc.tile_pool(name="z2p1", bufs=1))
                h1p1 = s2_es.enter_context(tc.tile_pool(name="h1p1", bufs=1))
                # seg-1 FFN runs in the tail (exp stream already over), so
                # gelu can be immediate: no h1pre tiles, no DVE stores.
                ffn1_pre, ffn1_gelu, ffn1_2 = ffn_fills(1, z2p1, h1p1, wfp, False,
                                                        rstd_on_act=True)
                # arm the gelu gate (final attention output has landed)
                nc.vector.tensor_scalar_mul(out=g.gelu_gate[:],
                                            in0=o_sb[1][KT - 1][:, SG - 1 : SG],
                                            scalar1=0.0)
                # Emission order matters (engines issue in-order): seg-0
                # gelus go first so ACT chews through them while PE runs
                # out-proj(1) + LN2(1) stats; FFN2(0) then fills the PE gap
                # left by the LN2(1) rstd/z ladder before FFN1(1) is ready.
                NLN = 2 * KT + 2  # ln_stat x6, ln_rows, ln_bcast, ln_z x6
                for f in ffn0_gelu:
                    f()
                for f in out_proj_fills(1, wfp) + ffn1_pre[:NLN]:
                    f()
                for f in ffn0_2 + ffn1_pre[NLN:] + ffn1_gelu + ffn1_2:
                    f()


def _tile_w(a, nk, w):
    """[K*128, NOUT] -> [NOUT//w, 128, nk, w] (kernel's stationary-tile order)."""
    kdim = a.shape[0]
    assert kdim == nk * P
    nj = a.shape[1] // w
    out = np.empty((nj, P, nk, w), dtype=a.dtype)
    for j in range(nj):
        blk = a[:, j * w : (j + 1) * w].reshape(nk, P, w)
        out[j] = blk.transpose(1, 0, 2)
    return np.ascontiguousarray(out)


def _tile_x(a, ck=CK):
    """[768, NCOLS] -> [NCOLS//ck, 128, KT, ck]."""
    d, ncols = a.shape
    nc_ = ncols // ck
    out = np.empty((nc_, P, KT, ck), dtype=a.dtype)
    for c in range(nc_):
        blk = a[:, c * ck : (c + 1) * ck].reshape(KT, P, ck)
        out[c] = blk.transpose(1, 0, 2)
    return np.ascontiguousarray(out)


def _tile_w_dr(a, s=W8S):
    """[K, NOUT] f32 -> [NOUT//128, 128, 4*(K//256), 128] dual-fp8 DoubleRow
    tiles, inner order (kp, ab, dr): w*s = wa + wb with both fp8."""
    import ml_dtypes

    f8 = ml_dtypes.float8_e4m3
    K, NOUT = a.shape
    nkp = K // (2 * P)
    nj = NOUT // P
    wa = (a * s).astype(f8)
    wb = (a * s - wa.astype(np.float32)).astype(f8)
    out = np.empty((nj, P, 4 * nkp, P), dtype=f8)
    for j in range(nj):
        for kp in range(nkp):
            for ab, w in ((0, wa), (1, wb)):
                blk = w[kp * 2 * P : (kp + 1) * 2 * P, j * P : (j + 1) * P]
                blk = blk.reshape(2, P, P)
                out[j, :, 4 * kp + 2 * ab + 0, :] = blk[0]
                out[j, :, 4 * kp + 2 * ab + 1, :] = blk[1]
    return np.ascontiguousarray(out)


def _prep_inputs(x, ln1_g, ln1_b, w_qkv, b_qkv, w_out, b_out, ln2_g, ln2_b, w1, b1, w2, b2):
    """Host-side prep: fold LN affines into weights, pre-tile, transpose x."""
    import ml_dtypes

    f32, bf = np.float32, ml_dtypes.bfloat16
    ln1_g = np.asarray(ln1_g, f32); ln1_b = np.asarray(ln1_b, f32)
    ln2_g = np.asarray(ln2_g, f32); ln2_b = np.asarray(ln2_b, f32)
    w_qkv = np.asarray(w_qkv, f32); w_out = np.asarray(w_out, f32)
    w1 = np.asarray(w1, f32); w2 = np.asarray(w2, f32)
    b_qkv = np.asarray(b_qkv, f32)

    wq_f = (ln1_g[:, None] * w_qkv[:, 0:D]).astype(f32)
    wk_f = (ln1_g[:, None] * w_qkv[:, D : 2 * D]).astype(f32)
    wv_f = (ln1_g[:, None] * w_qkv[:, 2 * D :]).astype(f32)
    wv_pad = np.zeros((D, 2 * CK), f32)
    wv_pad[:, :D] = wv_f

    common = {
        "wq": _tile_w(wq_f, KT, P),
        "wk": _tile_w(wk_f, KT, P),
        "wv": _tile_x(wv_pad, CK),  # same [c][p][k][ck] layout over dv chunks
        "wo": _tile_w(w_out.astype(bf), KT, P),
        "w1": _tile_w_dr(ln2_g[:, None] * w1),
        "w2": _tile_w_dr(w2),
        "bq": np.ascontiguousarray(ln1_b @ w_qkv[:, 0:D] + b_qkv[0:D]),
        "bv": np.ascontiguousarray(ln1_b @ w_qkv[:, 2 * D :] + b_qkv[2 * D :]),
        "bo": np.ascontiguousarray(np.asarray(b_out, f32)),
        "b1": np.ascontiguousarray(ln2_b @ w1 + np.asarray(b1, f32)),
        "b2": np.ascontiguousarray(np.asarray(b2, f32)),
        "ident": np.ascontiguousarray(np.eye(P, dtype=bf)),
        "onesd": np.ones((1, N), f32),
    }
    in_maps = []
    for c in range(8):
        b_idx, half = c // 2, c % 2
        xb = np.asarray(x[b_idx], dtype=f32)
        m = dict(common)
        xt = np.ascontiguousarray(xb.T)
        # own tokens first: softmax is invariant to key order, and this lets
        # one compiled program slice its own queries at columns 0:NQ
        xt_perm = np.concatenate(
            [xt[:, half * NQ : (half + 1) * NQ], xt[:, (1 - half) * NQ : (2 - half) * NQ]],
            axis=1)
        m["xT"] = _tile_x(np.ascontiguousarray(xt_perm))
        m["xTq2"] = np.ascontiguousarray(xt_perm[:, :NQ].astype(bf))
        in_maps.append(m)
    return in_maps


_NC_CACHE = {}


def _get_nc():
    if "nc" not in _NC_CACHE:
        _NC_CACHE["nc"] = build_nc()
    return _NC_CACHE["nc"]


def kernel(x, ln1_g, ln1_b, w_qkv, b_qkv, w_out, b_out, ln2_g, ln2_b, w1, b1, w2, b2,
           _trace=False, _tmpdir=None):
    in_maps = _prep_inputs(x, ln1_g, ln1_b, w_qkv, b_qkv, w_out, b_out,
                           ln2_g, ln2_b, w1, b1, w2, b2)
    nc = _get_nc()
    res = run_bass_kernel_spmd(nc, in_maps, list(range(8)), trace=_trace, tmpdir=_tmpdir)
    out = np.empty((B, N, D), dtype=np.float32)
    for c in range(8):
        b_idx, half = c // 2, c % 2
        out[b_idx, half * NQ : (half + 1) * NQ, :] = res.results[c]["yT"].T
    if _trace:
        return out, res
    return out



# revision 65
# speedup vs baseline: 1.0509x; 1.0509x over previous
"""Trainium2 Bass kernel for a dense transformer block (PreNorm attn + PreNorm MLP).

Sharding (8 cores, collective-free): core c -> batch b = c//2, sequence half
h = c%2.  Each core computes K/V for the full 2048-token sequence of its batch
element (redundant across the core pair) but Q/attention/FFN only for its own
1024 tokens.  The host permutes each core's token axis so the core's OWN 1024
tokens always occupy columns 0:1024 (softmax is invariant to key order), which
lets one compiled program serve all 8 cores with z_q a plain slice of z_full.

Layout: activations are feature-major ([feature, token]).  Weights are
host-pre-tiled so every DMA is one contiguous block; LN affines are folded into
downstream weights; LN stats come from ones-vector matmuls.

Attention is restructured around the cost model:
  * scores st[key, query] (f32r QK, 512-wide moving) -> exp on ACT over a
    2-head [128, 1024] PSUM tile -> AV with pexp as the STATIONARY operand:
    out[query, dv] = sum_k pexp[k, q] v[k, d].  This halves tensor-engine AV
    time vs the [dv+1, q] orientation (moving dim 65 vs 512 per key tile)
    and gives token-major AV output that is transposed back per 128x128
    block on the PE (free: Ldweights costs nothing, transpose 1cyc/row bf16).
  * the softmax denominator rides as an appended ones column of V (col 64),
    the per-query shift M rides as contraction row 64 (ones in kaug, -M in qt).
  * queries are processed in two 512-column segments over all 12 heads; the
    out-projection + LN2 + FFN of segment 0 overlaps segment 1's ACT-bound
    exp work, keeping the PE busy.  K (f32r kaug) is bounced through DRAM
    between segments instead of recomputed, as (lo, hi) half-sequence tiles
    so the reload cycles buffers at half-sequence granularity.
  * QK runs one t-step ahead of the exp stream so ACT rolls exp-to-exp.

FFN runs on fp8 DoubleRow matmuls (cost-model rate 0.5 cyc per output
column, 256-deep contraction): weights are dual-fp8 (w*64 = wa + wb, both
e4m3, residual-encoded; /64 folded into the gelu scale / bias adds).  FFN1
is three-term (za@wa + zb@wa + za@wb with z2 itself dual-fp8 from the LN2
writer, cross term dropped) at 0.75x bf16 cost; FFN2 contracts single-fp8
h1 (gelu writes e4m3 directly) against dual w2 at 0.5x.  Measured end-to-end
max rel err ~1.0e-2 vs the f32 reference (gate 2e-2).

Precision: score path (z, wq/wk, q, k, QK) in float32r; V/out bf16;
FFN dual-fp8 as above; x residual staged bf16.
"""

import sys

sys.path.insert(0, "/opt/trn_rl_repo")

import numpy as np

import concourse.bacc as bacc
import concourse.bass as bass
import concourse.tile as tile
from concourse import mybir
from concourse.bass_utils import run_bass_kernel_spmd

F32 = mybir.dt.float32
F32R = mybir.dt.float32r
BF16 = mybir.dt.bfloat16
FP8 = mybir.dt.float8e4
AF = mybir.ActivationFunctionType
ALU = mybir.AluOpType
DR = mybir.MatmulPerfMode.DoubleRow
W8S = 64.0  # fp8 weight pre-scale (keeps w out of the subnormal range)
RCP_W8S = 1.0 / W8S

D = 768
H = 12
HP = 6  # head pairs
DH = 64
F = 3072
B = 4
N = 2048
NQ = 1024  # tokens owned per core
P = 128
KT = D // P  # 6 feature k-tiles
MT = F // P  # 24 mlp-hidden tiles
NKT = N // P  # 16 key-token tiles
SG = 512  # query segment width
NSEG = NQ // SG  # 2
QB = SG // P  # 4 query blocks per segment
SCALE = float(DH**0.5)  # reference MULTIPLIES scores by sqrt(dh)
EXP_BIAS = -40.0  # pad on the exp argument (post-scale logit units)
SSTRIDE = 16  # key sampling stride for the shift estimate
NS = N // SSTRIDE
EPS = 1e-5
CK = 512


def build_nc():
    nc = bacc.Bacc("TRN2", target_bir_lowering=False, debug=False)

    xT = nc.dram_tensor("xT", [N // CK, P, KT, CK], F32R, kind="ExternalInput")
    xTq2 = nc.dram_tensor("xTq2", [D, NQ], BF16, kind="ExternalInput")
    wq = nc.dram_tensor("wq", [KT, P, KT, P], F32R, kind="ExternalInput")
    wk = nc.dram_tensor("wk", [KT, P, KT, P], F32R, kind="ExternalInput")
    wv = nc.dram_tensor("wv", [2, P, KT, CK], F32R, kind="ExternalInput")
    wo = nc.dram_tensor("wo", [KT, P, KT, P], BF16, kind="ExternalInput")
    # FFN weights: dual-fp8 DoubleRow tiles, inner dim order (kp, ab, dr)
    w1 = nc.dram_tensor("w1", [MT, P, 4 * (KT // 2), P], FP8, kind="ExternalInput")
    w2 = nc.dram_tensor("w2", [KT, P, 4 * (MT // 2), P], FP8, kind="ExternalInput")
    bq = nc.dram_tensor("bq", [D], F32, kind="ExternalInput")
    bv = nc.dram_tensor("bv", [D], F32R, kind="ExternalInput")
    bo = nc.dram_tensor("bo", [D], F32, kind="ExternalInput")
    b1 = nc.dram_tensor("b1", [F], F32, kind="ExternalInput")
    b2 = nc.dram_tensor("b2", [D], F32, kind="ExternalInput")
    ident = nc.dram_tensor("ident", [P, P], BF16, kind="ExternalInput")
    onesd = nc.dram_tensor("onesd", [1, N], F32R, kind="ExternalInput")
    yT = nc.dram_tensor("yT", [D, NQ], F32, kind="ExternalOutput")
    mscratch = nc.dram_tensor("mscratch", [H, NQ], F32R)
    qdram = nc.dram_tensor("qdram", [H, DH, SG], F32R)
    kdram = nc.dram_tensor("kdram", [H, DH + 1, N], F32R)

    with tile.TileContext(nc) as tc:
        _body(tc, xT, xTq2, wq, wk, wv, wo, w1, w2, bq, bv, bo, b1, b2,
              ident, onesd, yT, mscratch, qdram, kdram)
    nc.compile()
    return nc


class Ctx:
    pass


def _layernorm_fm(tc, g, load_fn, ncols, name, zpool, out_dt, wp, rstd_on_act=False,
                  lnps=None, on_chunk=None, z_tiles=None, rb=1, fp8_pair=None,
                  xsq_dve=False, m2f_dve=False):
    """Feature-major layernorm (affine folded into downstream weights).

    load_fn(k, c, sl) -> AP of a [128, CK] chunk of the input.
    Returns KT tiles [128, ncols] of dtype out_dt holding z = (x - mu) * rstd.
    With fp8_pair=(za, zb) (paired [128, 2, ncols] fp8 tiles) the result is
    written as a dual-fp8 pair instead: za = fp8(z), zb = fp8(z - za).
    """
    nc = tc.nc
    nch = ncols // CK
    ones_row = g.ones_row_r if out_dt == F32R else g.ones_row_b
    row_dt = F32R if out_dt == F32R else BF16

    if fp8_pair is not None:
        z_sb = None
    else:
        z_sb = z_tiles if z_tiles is not None else [
            zpool.tile([P, ncols], out_dt, name=f"{name}_z{k}") for k in range(KT)]
    for c in range(nch):
        sl = slice(c * CK, (c + 1) * CK)
        if lnps is not None:
            ps = lnps.tile([33, CK], F32, tag="lnst", bufs=2, name="lnst_ps")
        else:
            ps = g.psum_mm.tile([33, CK], F32, tag="mm", name="ln_ps")
        p1, p2 = ps[0:1, :], ps[32:33, :]
        for k in range(KT):
            xc = load_fn(k, c, sl)
            if xc.dtype == F32R:
                nc.tensor.matmul(p1[:], g.ones_col_r[:], xc,
                                 start=(k == 0), stop=(k == KT - 1))
            elif xc.dtype == BF16:
                nc.tensor.matmul(p1[:], g.ones_col[:], xc,
                                 start=(k == 0), stop=(k == KT - 1))
            else:
                xb = wp.tile([P, CK], BF16, tag="ln_xb")
                nc.vector.tensor_copy(out=xb[:], in_=xc)
                nc.tensor.matmul(p1[:], g.ones_col[:], xb[:],
                                 start=(k == 0), stop=(k == KT - 1))
            xsq = wp.tile([P, CK], BF16, tag="ln_xsq", bufs=2)
            if xsq_dve:
                nc.vector.tensor_mul(out=xsq[:], in0=xc, in1=xc)
            else:
                nc.scalar.activation(out=xsq[:], in_=xc, func=AF.Square)
            nc.tensor.matmul(p2[:], g.ones_col[:], xsq[:], start=(k == 0), stop=(k == KT - 1))
        s1 = wp.tile([1, CK], F32, name="s1r", tag="lnr_a", bufs=rb)
        s2 = wp.tile([1, CK], F32, name="s2r", tag="lnr_b", bufs=rb)
        rt = wp.tile([1, CK], F32, name="rtr", tag="lnr_c", bufs=1)
        y = wp.tile([1, CK], F32, name="yr", tag="lnr_y", bufs=1)
        nc.vector.tensor_scalar_mul(out=s1[:], in0=p1[:], scalar1=1.0 / D)
        nc.vector.tensor_scalar(out=s2[:], in0=p2[:], scalar1=1.0 / D,
                                scalar2=EPS, op0=ALU.mult, op1=ALU.add)
        nc.vector.tensor_mul(out=rt[:], in0=s1[:], in1=s1[:])  # mu^2
        nc.vector.tensor_sub(out=s2[:], in0=s2[:], in1=rt[:])  # var + eps
        if rstd_on_act:
            # rstd = exp(-0.5*ln(var)): fine where ACT is idle (preamble);
            # costs two act-table loads per chunk
            nc.scalar.activation(out=rt[:], in_=s2[:], func=AF.Ln)
            nc.scalar.activation(out=y[:], in_=rt[:], func=AF.Exp, scale=-0.5)
        else:
            # rstd = rsqrt(var) via Newton on DVE (vars are ~1, so a linear
            # seed converges in 3 steps); keeps LN2 off the ACT tables while
            # the attention exp stream runs
            nc.vector.tensor_scalar(out=y[:], in0=s2[:], scalar1=-0.5,
                                    scalar2=1.5, op0=ALU.mult, op1=ALU.add)
            for _ in range(3):
                nc.vector.tensor_mul(out=rt[:], in0=y[:], in1=y[:])  # y^2
                nc.vector.tensor_mul(out=rt[:], in0=rt[:], in1=s2[:])  # v*y^2
                nc.vector.tensor_scalar(out=rt[:], in0=rt[:], scalar1=-0.5,
                                        scalar2=1.5, op0=ALU.mult, op1=ALU.add)
                nc.vector.tensor_mul(out=y[:], in0=y[:], in1=rt[:])
        nc.vector.tensor_mul(out=s1[:], in0=s1[:], in1=y[:])  # m2 = mu*rstd
        rstd_r = wp.tile([1, CK], row_dt, name="rstdr", tag="lnr_d", bufs=2)
        nc.vector.tensor_copy(out=rstd_r[:], in_=y[:])
        m2_r = wp.tile([1, CK], row_dt, name="m2r", tag="lnr_e", bufs=2)
        nc.vector.tensor_copy(out=m2_r[:], in_=s1[:])

        if lnps is not None:
            rstdF = lnps.tile([P, CK], F32, tag="lnbc", bufs=4, name="rstdF")
            m2F = lnps.tile([P, CK], F32, tag="lnbc", bufs=4, name="m2F")
        else:
            rstdF = g.psum_mm.tile([P, CK], F32, tag="mm")
            m2F = g.psum_mm.tile([P, CK], F32, tag="mm")
        nc.tensor.matmul(rstdF[:], ones_row[:], rstd_r[:], start=True, stop=True)
        nc.tensor.matmul(m2F[:], ones_row[:], m2_r[:], start=True, stop=True)
        m2F_sb = wp.tile([P, CK], F32, tag="m2fsb", bufs=2)
        if m2f_dve:
            nc.vector.tensor_copy(out=m2F_sb[:], in_=m2F[:])
        else:
            nc.scalar.copy(out=m2F_sb[:], in_=m2F[:])
        for k in range(KT):
            xc = load_fn(k, c, sl)
            tmp = wp.tile([P, CK], F32, tag="lntmp", bufs=2)
            nc.vector.tensor_mul(out=tmp[:], in0=xc, in1=rstdF[:])
            if fp8_pair is None:
                nc.gpsimd.tensor_sub(out=z_sb[k][:, sl], in0=tmp[:], in1=m2F_sb[:])
            else:
                za, zb = fp8_pair
                a_ap = za[k // 2][:, k % 2, sl]
                nc.vector.tensor_sub(out=tmp[:], in0=tmp[:], in1=m2F_sb[:])
                nc.gpsimd.tensor_copy(out=a_ap, in_=tmp[:])
                nc.gpsimd.tensor_sub(out=zb[k // 2][:, k % 2, sl],
                                     in0=tmp[:], in1=a_ap)
        if on_chunk is not None:
            on_chunk(c)
    return z_sb


def _body(tc, xT, xTq2, wq, wk, wv, wo, w1, w2, bq, bv, bo, b1, b2,
          ident, onesd, yT, mscratch, qdram, kdram):
    nc = tc.nc
    from contextlib import ExitStack

    with ExitStack() as es:
        g = Ctx()
        g.singles = es.enter_context(tc.tile_pool(name="singles", bufs=1))
        g.rows = es.enter_context(tc.tile_pool(name="rows", bufs=1))
        g.work = es.enter_context(tc.tile_pool(name="work", bufs=2))
        g.wpool = es.enter_context(tc.tile_pool(name="wpool", bufs=2))
        # PSUM: mm [128,512] x2 up front; st/av created after the LN phase
        # (LN1 borrows their banks for chunk pipelining)
        g.psum_mm = es.enter_context(tc.tile_pool(name="psum_mm", bufs=2, space="PSUM"))

        x_pre = {}

        g.ones_col = g.singles.tile([P, 1], BF16, name="ones_col")
        nc.vector.memset(g.ones_col[:], 1.0)
        g.ones_col_f = g.singles.tile([P, 1], F32, name="ones_col_f")
        nc.vector.memset(g.ones_col_f[:], 1.0)
        g.ones_col_r = g.singles.tile([P, 1], F32R, name="ones_col_r")
        nc.vector.tensor_copy(out=g.ones_col_r[:], in_=g.ones_col_f[:])
        g.ones_row_b = g.singles.tile([1, P], BF16, name="ones_row_b")
        nc.vector.memset(g.ones_row_b[:], 1.0)
        g.ones_row_f = g.singles.tile([1, P], F32, name="ones_row_f")
        nc.vector.memset(g.ones_row_f[:], 1.0)
        g.ones_row_r = g.singles.tile([1, P], F32R, name="ones_row_r")
        nc.vector.tensor_copy(out=g.ones_row_r[:], in_=g.ones_row_f[:])
        g.eps_sb = g.singles.tile([1, 1], F32, name="eps")
        nc.vector.memset(g.eps_sb[:], EPS)
        g.expb_sb = g.singles.tile([P, 1], F32, name="expb")
        nc.vector.memset(g.expb_sb[:], EXP_BIAS)
        g.zeros_row = g.singles.tile([1, QB * (DH + 1)], BF16, name="zeros_row")
        nc.vector.memset(g.zeros_row[:], 0.0)
        g.gelu_gate = g.singles.tile([P, 1], F32, name="gelu_gate")
        g.ident = g.singles.tile([P, P], BF16, name="ident")

        def load_bias_cols(dram, n, name):
            t = g.singles.tile([P, n // P], F32, name=name)
            return t, lambda: nc.scalar.dma_start(
                out=t[:], in_=dram.ap().rearrange("(j p) -> p j", p=P))

        bo_sb, ld_bo = load_bias_cols(bo, D, "bo_sb")
        b1_sb, ld_b1 = load_bias_cols(b1, F, "b1_sb")
        b2_sb, ld_b2 = load_bias_cols(b2, D, "b2_sb")
        bq_sb, ld_bq = load_bias_cols(bq, D, "bq_sb")
        bv_row = g.singles.tile([1, D], F32R, name="bv_row")

        def emit_const_dmas():
            # emitted only after the first two x-chunk DMAs: the (shared)
            # DMA issue queue drains in emission order and the x stream is
            # the LN1 critical path.  These ride the ACT hwdge queue.
            nc.scalar.dma_start(out=g.ident[:], in_=ident.ap())
            nc.scalar.dma_start(out=bv_row[:],
                                in_=bv.ap().rearrange("(a n) -> a n", a=1))
            for ld in (ld_bq, ld_b1, ld_bo, ld_b2):
                ld()

        def stream_loader(dram, pool):
            state = {}
            def load(k, c, sl):
                if state.get("c") != c:
                    if c in x_pre:
                        state["t"] = x_pre[c]
                    else:
                        t = pool.tile([P, KT, CK], F32R, tag="xstream", name="xs")
                        nc.sync.dma_start(out=t[:], in_=dram.ap()[c])
                        state["t"] = t
                    state["c"] = c
                return state["t"][:, k, :]
            return load

        # ---------- persistent activation tiles ----------
        vpool = es.enter_context(tc.tile_pool(name="vpool", bufs=1))
        v_sb = [vpool.tile([P, H, DH + 1], BF16, name=f"v{t}") for t in range(NKT)]
        opool = es.enter_context(tc.tile_pool(name="opool", bufs=1))
        o_sb = [[opool.tile([P, SG], BF16, name=f"o0_{j}") for j in range(KT)], None]
        xmid = [None, None]  # filled per segment from scoped pools

        for t in range(NKT):
            nc.gpsimd.memset(v_sb[t][:], 1.0)  # col 64 of each head stays 1.0

        def w_load(dram, j, tag, dt=BF16, nk=KT, w=P, pool=None, bufs=None):
            t = (pool or g.wpool).tile([P, nk, w], dt, tag=tag, name=f"wt_{tag}{j}",
                                       **({"bufs": bufs} if bufs else {}))
            nc.sync.dma_start(out=t[:], in_=dram.ap()[j])
            return t


        def q_proj_pair(jp, z, cols, out_even, out_odd, spool, on_act=False):
            """Project the head pair jp's queries for z[:, cols].

            Both heads come out of one [128, SG] psum (full PE width).  The
            odd head's rows 64:128 are biased into a staging tile and DMA'd
            to out_odd (partition shift needs a DMA).  With on_act the bias
            adds run as ACT Identity-with-bias (keeps the preamble DVE queue
            clear) and the even head is written directly to its SBUF AP."""
            wqb = w_load(wq, jp, "wqk", dt=F32R, pool=wqkp)
            pt = g.psum_mm.tile([P, SG], F32, tag="mm")
            for k in range(KT):
                nc.tensor.matmul(pt[:], wqb[:, k, :], z[k][:, cols],
                                 start=(k == 0), stop=(k == KT - 1))
            stg = spool.tile([P, SG], F32R, tag="qstg")
            if on_act:
                nc.scalar.activation(out=out_even, in_=pt[0:DH, :],
                                     func=AF.Identity,
                                     bias=bq_sb[0:DH, jp : jp + 1], scale=1.0)
                nc.scalar.activation(out=stg[DH:P, :], in_=pt[DH:P, :],
                                     func=AF.Identity,
                                     bias=bq_sb[DH:P, jp : jp + 1], scale=1.0)
            else:
                nc.vector.tensor_scalar_add(out=stg[0:DH, :], in0=pt[0:DH, :],
                                            scalar1=bq_sb[0:DH, jp : jp + 1])
                nc.vector.tensor_scalar_add(out=stg[DH:P, :], in0=pt[DH:P, :],
                                            scalar1=bq_sb[DH:P, jp : jp + 1])
                nc.sync.dma_start(out=out_even, in_=stg[0:DH, :])
            nc.sync.dma_start(out=out_odd, in_=stg[DH:P, :])

        def m_shift(h, q_sb, seg):
            """Sampled row-max shift for head h, queries of segment seg.

            q_sb rows 0:64 hold the biased q.  Writes -max to mscratch[h, seg].
            kaugs[h] is an (lo, hi) pair of [DH+1, N/2] tiles; samples come
            half from each (one accumulation group, disjoint psum regions)."""
            m_sb = g.work.tile([P, QB], F32R, tag="msb")
            for qt_i in range(QB):
                ss = g.psum_mm.tile([P, CK], F32, tag="mm")
                for half in range(2):
                    ksamp = (kaugs[h][half][0:DH, :]
                             .rearrange("p (n t) -> p n t", t=SSTRIDE)[:, :, 0:1])
                    nc.tensor.matmul(ss[:, half * (NS // 2) : (half + 1) * (NS // 2)],
                                     q_sb[0:DH, qt_i * P : (qt_i + 1) * P],
                                     ksamp, start=(half == 0), stop=(half == 1),
                                     skip_group_check=True)
                nc.vector.tensor_reduce(
                    out=m_sb[:, qt_i : qt_i + 1], in_=ss[:, :NS],
                    axis=mybir.AxisListType.X, op=ALU.max, negate=True,
                )
            nc.sync.dma_start(
                out=mscratch.ap()[h : h + 1, seg * SG : (seg + 1) * SG]
                    .rearrange("a (c p) -> a p c", p=P),
                in_=m_sb[:],
            )

        def k_fills(jp, kpool):
            """Fine-grained filler closures for pair jp's K projection:
            [setup+chunk0, chunk1, chunk2, chunk3+kdram, shift-rows].

            kaug is stored as (lo, hi) [DH+1, N/2] tiles so the seg-1
            reload can cycle buffers at half-sequence granularity.  One
            [128, CK] psum per chunk covers both heads (full PE width); the
            odd head's rows 64:128 bounce through a staging tile + DMA."""
            NH = N // 2
            st = {}

            def chunk(c):
                ks, wkb = st["ks"], st["wkb"]
                half, co = c // 2, (c % 2) * CK
                osl = slice(co, co + CK)
                sl = slice(c * CK, (c + 1) * CK)
                pt = g.psum_mm.tile([P, CK], F32, tag="mm")
                for k in range(KT):
                    nc.tensor.matmul(pt[:], wkb[:, k, :], z_full[k][:, sl],
                                     start=(k == 0), stop=(k == KT - 1))
                nc.vector.tensor_copy(out=ks[0][half][0:DH, osl], in_=pt[0:DH, :])
                stg = qspool.tile([P, CK], F32R, tag="kstg", bufs=2)
                nc.vector.tensor_copy(out=stg[DH:P, :], in_=pt[DH:P, :])
                nc.sync.dma_start(out=ks[1][half][0:DH, osl], in_=stg[DH:P, :])

            def setup():
                ks = []
                for s in range(2):
                    h = 2 * jp + s
                    pair = []
                    for half in range(2):
                        kaug = kpool.tile([DH + 1, NH], F32R,
                                          name=f"kaug{h}_{half}", tag="kaug")
                        nc.sync.dma_start(out=kaug[DH : DH + 1, :],
                                          in_=onesd.ap()[0:1, 0:NH])
                        pair.append(kaug)
                    ks.append(tuple(pair))
                st["ks"] = ks
                st["wkb"] = w_load(wk, jp, "wqk", dt=F32R, pool=wqkp)
                chunk(0)

            def finish():
                chunk(3)
                for s in range(2):
                    for half in range(2):
                        nc.sync.dma_start(
                            out=kdram.ap()[2 * jp + s][:, half * NH : (half + 1) * NH],
                            in_=st["ks"][s][half][:])

            def shift():
                for s in range(2):
                    kaugs[2 * jp + s] = st["ks"][s]
                m_rows(jp)

            return [setup, lambda: chunk(1), lambda: chunk(2), finish, shift]

        def attn_pair(jp, seg, fillers, pexpool):
            """Attention for head pair jp over segment seg's 512 queries.

            fillers: list of zero-arg callables emitting independent PE work,
            interleaved into the t-loop to cover ACT-bound stretches."""
            h0, h1 = 2 * jp, 2 * jp + 1
            qts = (qt0[h0], qt0[h1]) if seg == 0 else (qt1s[h0], qt1s[h1])
            av = [g.psum_av.tile([P, QB * (DH + 1)], F32, tag="av", name=f"av{s}")
                  for s in range(2)]
            # The 4 query-block accumulation regions share one PSUM zero
            # region (2KB bank), so start_tensor_calc must fire exactly once
            # per bank: zero the whole tile with one K=1 matmul, then
            # accumulate with start=False.
            for s in range(2):
                nc.tensor.matmul(av[s][:], g.ones_row_b[:], g.zeros_row[:],
                                 start=True, stop=True)
            nfill = len(fillers)
            fi = 0

            def qk(t):
                st = g.psum_st.tile([P, 2 * SG], F32, tag="st")
                for s in range(2):
                    kh = kaugs[2 * jp + s][t // 8]
                    nc.tensor.matmul(st[:, s * SG : (s + 1) * SG],
                                     kh[:, (t % 8) * P : (t % 8 + 1) * P],
                                     qts[s][:], start=True, stop=True)
                return st

            # QK runs one step ahead of the exp stream: st(t+1) is issued
            # right after exp(t) so ACT rolls exp-to-exp without waiting on
            # PE, and fillers drain in the AV/exp shadow.
            st = qk(0)
            for t in range(NKT):
                pexp = pexpool.tile([P, 2 * SG], BF16, tag="pexp")
                nc.scalar.activation(out=pexp[:], in_=st[:], func=AF.Exp,
                                     scale=SCALE, bias=g.expb_sb[:])
                if t + 1 < NKT:
                    st = qk(t + 1)
                # interleave filler work so the PE queue stays fed while
                # exp(t) is still on ACT
                while fi * NKT < (t + 1) * nfill:
                    fillers[fi]()
                    fi += 1
                for s in range(2):
                    h = 2 * jp + s
                    for qb in range(QB):
                        nc.tensor.matmul(
                            av[s][:, qb * (DH + 1) : (qb + 1) * (DH + 1)],
                            pexp[:, s * SG + qb * P : s * SG + (qb + 1) * P],
                            v_sb[t][:, h, :],
                            start=False, stop=(t == NKT - 1),
                            skip_group_check=True)
            # normalize (token-major), then transpose pairs back to
            # feature-major o_sb via PE
            otok = g.work.tile([P, QB, P], BF16, tag="otok")
            rr = [g.work.tile([P, QB, 1], F32, tag="attn_r", bufs=4, name=f"r{s}")
                  for s in range(2)]
            for s in range(2):
                nc.vector.reciprocal(
                    out=rr[s][:],
                    in_=av[s][:].rearrange("p (q c) -> p q c", c=DH + 1)[:, :, DH : DH + 1])
            for qb in range(QB):
                for s in range(2):
                    nc.vector.tensor_scalar_mul(
                        out=otok[:, qb, s * DH : (s + 1) * DH],
                        in0=av[s][:, qb * (DH + 1) : qb * (DH + 1) + DH],
                        scalar1=rr[s][:, qb, :])
            for qb in range(QB):
                ptr = g.psum_av.tile([P, P], BF16, tag="av", name="ptr")
                nc.tensor.transpose(ptr[:], otok[:, qb, :], g.ident[:])
                nc.vector.tensor_copy(out=o_sb[seg][jp][:, qb * P : (qb + 1) * P],
                                      in_=ptr[:])

        def out_proj_fills(seg, wfp):
            """Closures: o_sb[seg] @ wo + bo + x residual -> xmid[seg]."""
            xq2 = [wfp.tile([P, SG], BF16, tag="xq2", bufs=6, name=f"xq2_{seg}_{k}")
                   for k in range(KT)]

            def xq2_load():
                for k in range(KT):
                    nc.sync.dma_start(
                        out=xq2[k][:],
                        in_=xTq2.ap()[k * P : (k + 1) * P, seg * SG : (seg + 1) * SG])

            def op_j(j):
                pt = g.psum_mm.tile([P, SG], F32, tag="mm")
                for k in range(KT):
                    nc.tensor.matmul(pt[:], wobs[j][:, k, :], o_sb[seg][k][:],
                                     start=(k == 0), stop=(k == KT - 1))
                tmp = wfp.tile([P, SG], F32, tag="tmpf4")
                nc.vector.tensor_scalar_add(out=tmp[:], in0=pt[:],
                                            scalar1=bo_sb[:, j : j + 1])
                nc.vector.tensor_add(out=xmid[seg][j][:], in0=tmp[:], in1=xq2[j][:])

            return [xq2_load] + [lambda j=j: op_j(j) for j in range(KT)]

        def ffn_fills(seg, z2pool, h1pool, wfp, defer_gelu, rstd_on_act=False):
            """Closure groups for LN2 + FFN over segment seg's tokens.

            FFN1 runs as dual-fp8 DoubleRow (three-term: za@wa + zb@wa +
            za@wb, cross term dropped); FFN2 as single-fp8 h1 against dual
            fp8 w2.  Weights carry a x64 pre-scale, corrected in the gelu
            scale / bias adds.  With defer_gelu, ffn1 stores biased pre-gelu
            h1 via DVE (so no Gelu touches ACT while the attention exp
            stream is running) and the returned gelu fills apply Gelu
            later, writing the fp8 h1 tiles.  Returns
            (pre_fills, gelu_fills, ffn2_fills)."""
            KP = KT // 2
            MP = MT // 2
            z2a = [z2pool.tile([P, 2, SG], FP8, name=f"z2a_{seg}_{kp}")
                   for kp in range(KP)]
            z2b = [z2pool.tile([P, 2, SG], FP8, name=f"z2b_{seg}_{kp}")
                   for kp in range(KP)]
            h1q = [h1pool.tile([P, 2, SG], FP8, name=f"h1_{seg}_{mp}")
                   for mp in range(MP)]
            h1pre = ([h1pool.tile([P, SG], BF16, name=f"h1p_{seg}_{m}")
                      for m in range(MT)] if defer_gelu else None)

            # LN2 as fine-grained closures (so the seg-1 filler interleave
            # spreads its DVE/PE load instead of spiking).  xsq + m2F stay
            # off ACT: LN2(0) runs inside the ACT-bound exp stream, LN2(1)
            # runs while ACT drains the gelu backlog.
            lns = {}

            def ln_stat(k):
                if k == 0:
                    lns["ps"] = g.psum_mm.tile([33, SG], F32, tag="mm",
                                               name="ln2_ps")
                ps = lns["ps"]
                xc = xmid[seg][k][:]
                nc.tensor.matmul(ps[0:1, :], g.ones_col[:], xc,
                                 start=(k == 0), stop=(k == KT - 1))
                xsq = wfp.tile([P, SG], BF16, tag="ln_xsq", bufs=2)
                nc.vector.tensor_mul(out=xsq[:], in0=xc, in1=xc)
                nc.tensor.matmul(ps[32:33, :], g.ones_col[:], xsq[:],
                                 start=(k == 0), stop=(k == KT - 1))

            def ln_rows():
                ps = lns["ps"]
                s1 = wfp.tile([1, SG], F32, name="s1r", tag="lnr_a", bufs=1)
                s2 = wfp.tile([1, SG], F32, name="s2r", tag="lnr_b", bufs=1)
                rt = wfp.tile([1, SG], F32, name="rtr", tag="lnr_c", bufs=1)
                y = wfp.tile([1, SG], F32, name="yr", tag="lnr_y", bufs=1)
                nc.vector.tensor_scalar_mul(out=s1[:], in0=ps[0:1, :],
                                            scalar1=1.0 / D)
                nc.vector.tensor_scalar(out=s2[:], in0=ps[32:33, :],
                                        scalar1=1.0 / D, scalar2=EPS,
                                        op0=ALU.mult, op1=ALU.add)
                nc.vector.tensor_mul(out=rt[:], in0=s1[:], in1=s1[:])
                nc.vector.tensor_sub(out=s2[:], in0=s2[:], in1=rt[:])
                if rstd_on_act:
                    nc.scalar.activation(out=rt[:], in_=s2[:], func=AF.Ln)
                    nc.scalar.activation(out=y[:], in_=rt[:], func=AF.Exp,
                                         scale=-0.5)
                else:
                    nc.vector.tensor_scalar(out=y[:], in0=s2[:], scalar1=-0.5,
                                            scalar2=1.5, op0=ALU.mult,
                                            op1=ALU.add)
                    for _ in range(3):
                        nc.vector.tensor_mul(out=rt[:], in0=y[:], in1=y[:])
                        nc.vector.tensor_mul(out=rt[:], in0=rt[:], in1=s2[:])
                        nc.vector.tensor_scalar(out=rt[:], in0=rt[:],
                                                scalar1=-0.5, scalar2=1.5,
                                                op0=ALU.mult, op1=ALU.add)
                        nc.vector.tensor_mul(out=y[:], in0=y[:], in1=rt[:])
                nc.vector.tensor_mul(out=s1[:], in0=s1[:], in1=y[:])
                rstd_r = wfp.tile([1, SG], BF16, name="rstdr", tag="lnr_d", bufs=1)
                nc.vector.tensor_copy(out=rstd_r[:], in_=y[:])
                m2_r = wfp.tile([1, SG], BF16, name="m2r", tag="lnr_e", bufs=1)
                nc.vector.tensor_copy(out=m2_r[:], in_=s1[:])
                lns["rstd_r"], lns["m2_r"] = rstd_r, m2_r

            def ln_bcast():
                rstdF = g.psum_mm.tile([P, SG], F32, tag="mm")
                m2F = g.psum_mm.tile([P, SG], F32, tag="mm")
                nc.tensor.matmul(rstdF[:], g.ones_row_b[:], lns["rstd_r"][:],
                                 start=True, stop=True)
                nc.tensor.matmul(m2F[:], g.ones_row_b[:], lns["m2_r"][:],
                                 start=True, stop=True)
                m2F_sb = wfp.tile([P, SG], F32, tag="m2fsb", bufs=2)
                nc.vector.tensor_copy(out=m2F_sb[:], in_=m2F[:])
                lns["rstdF"], lns["m2F_sb"] = rstdF, m2F_sb

            def ln_z(k):
                xc = xmid[seg][k][:]
                tmp = wfp.tile([P, SG], F32, tag="lntmp", bufs=2)
                nc.vector.tensor_mul(out=tmp[:], in0=xc, in1=lns["rstdF"][:])
                nc.vector.tensor_sub(out=tmp[:], in0=tmp[:], in1=lns["m2F_sb"][:])
                a_ap = z2a[k // 2][:, k % 2, :]
                nc.gpsimd.tensor_copy(out=a_ap, in_=tmp[:])
                nc.gpsimd.tensor_sub(out=z2b[k // 2][:, k % 2, :],
                                     in0=tmp[:], in1=a_ap)

            ln_fills = ([lambda k=k: ln_stat(k) for k in range(KT)]
                        + [ln_rows, ln_bcast]
                        + [lambda k=k: ln_z(k) for k in range(KT)])

            def ffn1(m):
                w1b = w_load(w1, m, "w1", dt=FP8, nk=4 * KP, pool=wfp, bufs=3)
                pt = g.psum_mm.tile([P, SG], F32, tag="mm")
                terms = []
                for kp in range(KP):
                    wa = w1b[:, 4 * kp : 4 * kp + 2, :]
                    wb = w1b[:, 4 * kp + 2 : 4 * kp + 4, :]
                    terms += [(wa, z2a[kp]), (wa, z2b[kp]), (wb, z2a[kp])]
                for i, (wv, zv) in enumerate(terms):
                    nc.tensor.matmul(pt[:], wv, zv[:], start=(i == 0),
                                     stop=(i == len(terms) - 1), perf_mode=DR)
                if defer_gelu:
                    nc.vector.tensor_scalar(out=h1pre[m][:], in0=pt[:],
                                            scalar1=RCP_W8S,
                                            scalar2=b1_sb[:, m : m + 1],
                                            op0=ALU.mult, op1=ALU.add)
                else:
                    nc.scalar.activation(out=h1q[m // 2][:, m % 2, :], in_=pt[:],
                                         func=AF.Gelu, bias=b1_sb[:, m : m + 1],
                                         scale=RCP_W8S)

            def gelu(m):
                # bias is numerically zero but reads the gate tile, which is
                # written only after the last attention output lands: all
                # gelus become ready together, after the exp stream, so the
                # act-table pass switches tables once instead of thrashing
                nc.scalar.activation(out=h1q[m // 2][:, m % 2, :],
                                     in_=h1pre[m][:], func=AF.Gelu,
                                     bias=g.gelu_gate[:], scale=1.0)

            def ffn2(j):
                w2b = w_load(w2, j, "w2", dt=FP8, nk=4 * MP, pool=wfp, bufs=3)
                pt = g.psum_mm.tile([P, SG], F32, tag="mm")
                for i in range(2 * MP):
                    mp, ab = i // 2, i % 2
                    base = 4 * mp + 2 * ab
                    nc.tensor.matmul(pt[:], w2b[:, base : base + 2, :],
                                     h1q[mp][:], start=(i == 0),
                                     stop=(i == 2 * MP - 1), perf_mode=DR)
                tmp = wfp.tile([P, SG], F32, tag="tmpf")
                nc.vector.tensor_scalar(out=tmp[:], in0=pt[:], scalar1=RCP_W8S,
                                        scalar2=b2_sb[:, j : j + 1],
                                        op0=ALU.mult, op1=ALU.add)
                out_t = wfp.tile([P, SG], F32, tag="f2_out")
                nc.vector.tensor_add(out=out_t[:], in0=tmp[:], in1=xmid[seg][j][:])
                nc.sync.dma_start(
                    out=yT.ap()[j * P : (j + 1) * P, seg * SG : (seg + 1) * SG],
                    in_=out_t[:])

            pre = ln_fills + [lambda m=m: ffn1(m) for m in range(MT)]
            gelus = [] if not defer_gelu else [lambda m=m: gelu(m) for m in range(MT)]
            return pre, gelus, [lambda j=j: ffn2(j) for j in range(KT)]

        # ================= preamble + segment 0 =================
        kaugs = {}
        with ExitStack() as pre_es:
            zpool = pre_es.enter_context(tc.tile_pool(name="z", bufs=1))
            z_full = [zpool.tile([P, N], F32R, name=f"ln1_z{k}") for k in range(KT)]
            qpool0 = pre_es.enter_context(tc.tile_pool(name="qpool0", bufs=1))
            qt0 = [qpool0.tile([DH + 1, SG], F32R, name=f"qt0_{h}") for h in range(H)]
            wqkp = pre_es.enter_context(tc.tile_pool(name="wqkp", bufs=2))
            qspool = pre_es.enter_context(tc.tile_pool(name="qspool", bufs=2))
            vwp = pre_es.enter_context(tc.tile_pool(name="vwp", bufs=1))
            kp0 = pre_es.enter_context(tc.tile_pool(name="kp0", bufs=4))

            vwb_cache = {}

            def v_proj(ci, half, ts, on_act):
                """V projection for a 256-wide dv quarter, key tiles ts.
                The bias rides as a K=1 ones-row matmul so the psum->SBUF
                move is a plain copy, placeable on ACT (preamble) or DVE."""
                HW = CK // 2
                lo = ci * CK + half * HW
                w = min(D, lo + HW) - lo
                if w <= 0:
                    return
                key = (ci, half)
                if key not in vwb_cache:
                    wvb = vwp.tile([P, KT, HW], F32R, tag="wv",
                                   name=f"wvb{ci}_{half}", bufs=2)
                    nc.sync.dma_start(
                        out=wvb[:], in_=wv.ap()[ci][:, :, half * HW : (half + 1) * HW])
                    vwb_cache[key] = wvb
                wvb = vwb_cache[key]
                h0 = lo // DH
                nh = w // DH
                for t in ts:
                    pt = g.psum_mm.tile([P, HW], F32, tag="mm", name="vpt")
                    for k in range(KT):
                        nc.tensor.matmul(pt[:, :w],
                                         z_full[k][:, t * P : (t + 1) * P],
                                         wvb[:, k, :w],
                                         start=(k == 0), stop=False)
                    nc.tensor.matmul(pt[:, :w], g.ones_row_r[:],
                                     bv_row[0:1, lo : lo + w],
                                     start=False, stop=True)
                    src = pt[:, :w].rearrange("p (h d) -> p h d", d=DH)
                    dst = v_sb[t][:, h0 : h0 + nh, 0:DH]
                    if on_act:
                        nc.scalar.copy(out=dst, in_=src)
                    else:
                        nc.vector.tensor_copy(out=dst, in_=src)

            # x chunks 0/1 are the first DMAs emitted (LN1 critical path);
            # everything else queues behind them
            ln_xsp_cm = tc.tile_pool(name="xsp", bufs=2)
            xsp = ln_xsp_cm.__enter__()
            for c in (0, 1):
                t = xsp.tile([P, KT, CK], F32R, tag="xstream", name="xs")
                nc.sync.dma_start(out=t[:], in_=xT.ap()[c])
                x_pre[c] = t
            emit_const_dmas()

            # pair-0 K runs inside the LN chunk callback so the first QK can
            # start right after the last z chunk lands
            NH = N // 2
            kaug_p0 = []
            for h in range(2):
                pair = []
                for half in range(2):
                    kaug = kp0.tile([DH + 1, NH], F32R,
                                    name=f"kaug_p0_{h}_{half}", tag="kaug")
                    nc.sync.dma_start(out=kaug[DH : DH + 1, :],
                                      in_=onesd.ap()[0:1, 0:NH])
                    pair.append(kaug)
                kaug_p0.append(tuple(pair))
                kaugs[h] = kaug_p0[h]
            wkb0 = w_load(wk, 0, "wqk", dt=F32R, pool=wqkp)

            def ln_chunk_cb(c):
                if c == 1:
                    # z chunk 0 is live: queries(seg0) for all heads
                    for jp in range(HP):
                        q_proj_pair(jp, z_full, slice(0, SG),
                                    qt0[2 * jp][0:DH, :], qt0[2 * jp + 1][0:DH, :],
                                    qspool, on_act=True)
                half, co = c // 2, (c % 2) * CK
                osl = slice(co, co + CK)
                sl = slice(c * CK, (c + 1) * CK)
                pt = g.psum_mm.tile([P, CK], F32, tag="mm", name="kpt")
                for k in range(KT):
                    nc.tensor.matmul(pt[:], wkb0[:, k, :], z_full[k][:, sl],
                                     start=(k == 0), stop=(k == KT - 1))
                nc.vector.tensor_copy(out=kaug_p0[0][half][0:DH, osl],
                                      in_=pt[0:DH, :])
                stg = qspool.tile([P, CK], F32R, tag="kstg", bufs=2)
                nc.vector.tensor_copy(out=stg[DH:P, :], in_=pt[DH:P, :])
                nc.sync.dma_start(out=kaug_p0[1][half][0:DH, osl],
                                  in_=stg[DH:P, :])
                v_proj(0, 0, range(4 * c, 4 * c + 4), True)  # heads 0-3

            with ExitStack() as ln_es:
                lnw1 = ln_es.enter_context(tc.tile_pool(name="lnw1", bufs=1))
                lnps = ln_es.enter_context(tc.tile_pool(name="lnps", bufs=1,
                                                        space="PSUM"))
                _layernorm_fm(tc, g, stream_loader(xT, xsp), N, "ln1",
                              zpool, F32R, wp=lnw1, rstd_on_act=True,
                              lnps=lnps, on_chunk=ln_chunk_cb, z_tiles=z_full,
                              rb=2)
            ln_xsp_cm.__exit__(None, None, None)
            g.psum_st = es.enter_context(
                tc.tile_pool(name="psum_st", bufs=2, space="PSUM"))
            g.psum_av = es.enter_context(
                tc.tile_pool(name="psum_av", bufs=2, space="PSUM"))

            for s in range(2):
                for half in range(2):
                    nc.sync.dma_start(
                        out=kdram.ap()[s][:, half * NH : (half + 1) * NH],
                        in_=kaug_p0[s][half][:])

            kpool_a = pre_es.enter_context(tc.tile_pool(name="kpool_a", bufs=8))
            pexp_a = pre_es.enter_context(tc.tile_pool(name="pexp_a", bufs=5))

            def m_rows(jpn):
                for h in (2 * jpn, 2 * jpn + 1):
                    m_shift(h, qt0[h], 0)
                    nc.sync.dma_start(out=qt0[h][DH : DH + 1, :],
                                      in_=mscratch.ap()[h : h + 1, 0:SG])

            m_rows(0)

            qt1s = {}

            def load_pair(jp):
                """Stream kaug + q back for pair jp, compute seg-1 shift."""
                NH = N // 2
                for s in range(2):
                    h = 2 * jp + s
                    pair = []
                    for half in range(2):
                        kaug = kpool_b.tile([DH + 1, NH], F32R,
                                            name=f"kaug1_{h}_{half}",
                                            tag="kaug")
                        nc.sync.dma_start(
                            out=kaug[:],
                            in_=kdram.ap()[h][:, half * NH : (half + 1) * NH])
                        pair.append(kaug)
                    kaugs[h] = tuple(pair)
                    qt = qpool1.tile([DH + 1, SG], F32R, tag="qt1")
                    nc.sync.dma_start(out=qt[0:DH, :], in_=qdram.ap()[h])
                    m_shift(h, qt, 1)
                    nc.sync.dma_start(out=qt[DH : DH + 1, :],
                                      in_=mscratch.ap()[h : h + 1, SG:NQ])
                    qt1s[h] = qt

            # ============== segment 0 ==============
            # Fine-grained fillers, biased toward late pairs so the exp
            # stream's ACT backlog can drain while PE still has work:
            #   jp0-4: K(jp+1) in 5 chunks; jp0-3: one V quarter-half in
            #   two 4-tile chunks (V(q) completes one pair before its
            #   consumer); jp4-5: the Q(seg1) projections.
            def q1_fill(jp):
                # Q(seg1) goes straight to DRAM from the psum staging
                # tile; its shift row is computed in segment 1.
                return lambda: q_proj_pair(
                    jp, z_full, slice(SG, NQ),
                    qdram.ap()[2 * jp], qdram.ap()[2 * jp + 1], qspool)

            def seg0_filler(jp):
                fills = []
                if jp + 1 < HP:
                    fills += k_fills(jp + 1, kpool_a)
                if jp < 4:
                    ci, half = (0, 1) if jp < 2 else (1, 0)
                    lo = (jp % 2) * 8
                    for o in (0, 4):
                        fills.append(lambda ci=ci, half=half, lo=lo + o:
                                     v_proj(ci, half, range(lo, lo + 4), False))
                if jp == 4:
                    fills += [q1_fill(0), q1_fill(1)]
                elif jp == 5:
                    fills += [q1_fill(jpq) for jpq in range(2, HP)]
                return fills

            for jp in range(HP):
                attn_pair(jp, 0, seg0_filler(jp), pexp_a)
        # z_full + qt0 + seg0 kaug/pexp pools die here

        # ============== segment 1 (+ seg-0 FFN as filler) ==============
        with ExitStack() as late_es:
            opool1 = late_es.enter_context(tc.tile_pool(name="opool1", bufs=1))
            o_sb[1] = [opool1.tile([P, SG], BF16, name=f"o1_{j}") for j in range(KT)]
            wop = late_es.enter_context(tc.tile_pool(name="wop", bufs=1))
            # wo loads ride a lazy closure so their 6 DMAs queue after the
            # boundary-critical kaug/qt reloads, not before
            wobs = []

            def wo_load():
                wobs.extend(w_load(wo, j, f"wo{j}", pool=wop) for j in range(KT))

            xmpool = late_es.enter_context(tc.tile_pool(name="xmpool", bufs=1))
            xmid[0] = [xmpool.tile([P, SG], BF16, name=f"xm0_{j}") for j in range(KT)]
            xmid[1] = [xmpool.tile([P, SG], BF16, name=f"xm1_{j}") for j in range(KT)]

            h1p0 = late_es.enter_context(tc.tile_pool(name="h1p0", bufs=1))
            wfp = late_es.enter_context(tc.tile_pool(name="wfp", bufs=2))
            with ExitStack() as s1_es:
                z2p0 = s1_es.enter_context(tc.tile_pool(name="z2p0", bufs=1))
                qpool1 = s1_es.enter_context(tc.tile_pool(name="qpool1", bufs=4))
                kpool_b = s1_es.enter_context(tc.tile_pool(name="kpool_b", bufs=6))
                pexp_b = s1_es.enter_context(tc.tile_pool(name="pexp_b", bufs=4))

                load_pair(0)
                wo_load()

                # fillers: prefetches, then out-proj(seg0) + LN2(seg0) + the
                # FFN1(seg0) matmuls (gelu deferred to the tail so ACT stays
                # on the Exp table throughout the attention stream)
                ffn0_pre, ffn0_gelu, ffn0_2 = ffn_fills(0, z2p0, h1p0, wfp, True)
                fills_all = out_proj_fills(0, wfp) + ffn0_pre
                per = (len(fills_all) + HP - 1) // HP
                for jp in range(HP):
                    fills = []
                    if jp + 1 < HP:
                        fills.append(lambda jpn=jp + 1: load_pair(jpn))
                    fills += fills_all[jp * per : (jp + 1) * per]
                    attn_pair(jp, 1, fills, pexp_b)

            # ====== tail: gelu(seg0) + FFN2(seg0) + out-proj/FFN(seg1) ======
            with ExitStack() as s2_es:
                z2p1 = s2_es.enter_context(tc.tile_pool(name="z2p1", bufs=1))
                h1p1 = s2_es.enter_context(tc.tile_pool(name="h1p1", bufs=1))
                # seg-1 FFN runs in the tail (exp stream already over), so
                # gelu can be immediate: no h1pre tiles, no DVE stores.
                ffn1_pre, ffn1_gelu, ffn1_2 = ffn_fills(1, z2p1, h1p1, wfp, False,
                                                        rstd_on_act=True)
                # arm the gelu gate (final attention output has landed)
                nc.vector.tensor_scalar_mul(out=g.gelu_gate[:],
                                            in0=o_sb[1][KT - 1][:, SG - 1 : SG],
                                            scalar1=0.0)
                # Emission order matters (engines issue in-order): seg-0
                # gelus go first so ACT chews through them while PE runs
                # out-proj(1) + LN2(1) stats; FFN2(0) then fills the PE gap
                # left by the LN2(1) rstd/z ladder before FFN1(1) is ready.
                NLN = 2 * KT + 2  # ln_stat x6, ln_rows, ln_bcast, ln_z x6
                for f in ffn0_gelu:
                    f()
                for f in out_proj_fills(1, wfp) + ffn1_pre[:NLN]:
                    f()
                for f in ffn0_2 + ffn1_pre[NLN:] + ffn1_gelu + ffn1_2:
                    f()


def _tile_w(a, nk, w):
    """[K*128, NOUT] -> [NOUT//w, 128, nk, w] (kernel's stationary-tile order)."""
    kdim = a.shape[0]
    assert kdim == nk * P
    nj = a.shape[1] // w
    out = np.empty((nj, P, nk, w), dtype=a.dtype)
    for j in range(nj):
        blk = a[:, j * w : (j + 1) * w].reshape(nk, P, w)
        out[j] = blk.transpose(1, 0, 2)
    return np.ascontiguousarray(out)


def _tile_x(a, ck=CK):
    """[768, NCOLS] -> [NCOLS//ck, 128, KT, ck]."""
    d, ncols = a.shape
    nc_ = ncols // ck
    out = np.empty((nc_, P, KT, ck), dtype=a.dtype)
    for c in range(nc_):
        blk = a[:, c * ck : (c + 1) * ck].reshape(KT, P, ck)
        out[c] = blk.transpose(1, 0, 2)
    return np.ascontiguousarray(out)


def _tile_w_dr(a, s=W8S):
    """[K, NOUT] f32 -> [NOUT//128, 128, 4*(K//256), 128] dual-fp8 DoubleRow
    tiles, inner order (kp, ab, dr): w*s = wa + wb with both fp8."""
    import ml_dtypes

    f8 = ml_dtypes.float8_e4m3
    K, NOUT = a.shape
    nkp = K // (2 * P)
    nj = NOUT // P
    wa = (a * s).astype(f8)
    wb = (a * s - wa.astype(np.float32)).astype(f8)
    out = np.empty((nj, P, 4 * nkp, P), dtype=f8)
    for j in range(nj):
        for kp in range(nkp):
            for ab, w in ((0, wa), (1, wb)):
                blk = w[kp * 2 * P : (kp + 1) * 2 * P, j * P : (j + 1) * P]
                blk = blk.reshape(2, P, P)
                out[j, :, 4 * kp + 2 * ab + 0, :] = blk[0]
                out[j, :, 4 * kp + 2 * ab + 1, :] = blk[1]
    return np.ascontiguousarray(out)


def _prep_inputs(x, ln1_g, ln1_b, w_qkv, b_qkv, w_out, b_out, ln2_g, ln2_b, w1, b1, w2, b2):
    """Host-side prep: fold LN affines into weights, pre-tile, transpose x."""
    import ml_dtypes

    f32, bf = np.float32, ml_dtypes.bfloat16
    ln1_g = np.asarray(ln1_g, f32); ln1_b = np.asarray(ln1_b, f32)
    ln2_g = np.asarray(ln2_g, f32); ln2_b = np.asarray(ln2_b, f32)
    w_qkv = np.asarray(w_qkv, f32); w_out = np.asarray(w_out, f32)
    w1 = np.asarray(w1, f32); w2 = np.asarray(w2, f32)
    b_qkv = np.asarray(b_qkv, f32)

    wq_f = (ln1_g[:, None] * w_qkv[:, 0:D]).astype(f32)
    wk_f = (ln1_g[:, None] * w_qkv[:, D : 2 * D]).astype(f32)
    wv_f = (ln1_g[:, None] * w_qkv[:, 2 * D :]).astype(f32)
    wv_pad = np.zeros((D, 2 * CK), f32)
    wv_pad[:, :D] = wv_f

    common = {
        "wq": _tile_w(wq_f, KT, P),
        "wk": _tile_w(wk_f, KT, P),
        "wv": _tile_x(wv_pad, CK),  # same [c][p][k][ck] layout over dv chunks
        "wo": _tile_w(w_out.astype(bf), KT, P),
        "w1": _tile_w_dr(ln2_g[:, None] * w1),
        "w2": _tile_w_dr(w2),
        "bq": np.ascontiguousarray(ln1_b @ w_qkv[:, 0:D] + b_qkv[0:D]),
        "bv": np.ascontiguousarray(ln1_b @ w_qkv[:, 2 * D :] + b_qkv[2 * D :]),
        "bo": np.ascontiguousarray(np.asarray(b_out, f32)),
        "b1": np.ascontiguousarray(ln2_b @ w1 + np.asarray(b1, f32)),
        "b2": np.ascontiguousarray(np.asarray(b2, f32)),
        "ident": np.ascontiguousarray(np.eye(P, dtype=bf)),
        "onesd": np.ones((1, N), f32),
    }
    in_maps = []
    for c in range(8):
        b_idx, half = c // 2, c % 2
        xb = np.asarray(x[b_idx], dtype=f32)
        m = dict(common)
        xt = np.ascontiguousarray(xb.T)
        # own tokens first: softmax is invariant to key order, and this lets
        # one compiled program slice its own queries at columns 0:NQ
        xt_perm = np.concatenate(
            [xt[:, half * NQ : (half + 1) * NQ], xt[:, (1 - half) * NQ : (2 - half) * NQ]],
            axis=1)
        m["xT"] = _tile_x(np.ascontiguousarray(xt_perm))
        m["xTq2"] = np.ascontiguousarray(xt_perm[:, :NQ].astype(bf))
        in_maps.append(m)
    return in_maps


_NC_CACHE = {}


def _get_nc():
    if "nc" not in _NC_CACHE:
        _NC_CACHE["nc"] = build_nc()
    return _NC_CACHE["nc"]


def kernel(x, ln1_g, ln1_b, w_qkv, b_qkv, w_out, b_out, ln2_g, ln2_b, w1, b1, w2, b2,
           _trace=False, _tmpdir=None):
    in_maps = _prep_inputs(x, ln1_g, ln1_b, w_qkv, b_qkv, w_out, b_out,
                           ln2_g, ln2_b, w1, b1, w2, b2)
    nc = _get_nc()
    res = run_bass_kernel_spmd(nc, in_maps, list(range(8)), trace=_trace, tmpdir=_tmpdir)
    out = np.empty((B, N, D), dtype=np.float32)
    for c in range(8):
        b_idx, half = c // 2, c % 2
        out[b_idx, half * NQ : (half + 1) * NQ, :] = res.results[c]["yT"].T
    if _trace:
        return out, res
    return out

